# revision 7
# baseline (speedup 1.0000x reference)
"""Self-contained Trainium2 Bass kernel for the AttnBlock problem — fp8 edition.

Sharding: 8 cores; core c handles batch b = c//2, query rows
[qh*1152, (qh+1)*1152) with qh = c%2.  Each core computes full K/V for its
batch (duplicated across the 2 cores of a batch) so there are NO collectives.
Per-core token order is rotated so each core's own query tokens are always
columns 0..1151.

All heavy matmuls run in fp8e4m3 with DoubleRow perf mode (2 contraction
subtiles per pass, 0.5 cycles/column).  Weights are host-scaled by 32 into
the fp8 sweet spot; activations carry powers-of-two scales that are folded
into downstream scalars:
  qrotF/krotF = fp8(32*rope(q|k)), scores PSUM = 8192*s (s = q.k/sqrt(64))
  exp: 8*exp(s) as fp8 — on ACT (Exp table, scale 1/8192, bias ln8) or on
  DVE via the direct fp8-byte trick u8 = round(8*log2(e)*s + 80).
  v_allF = fp8(32*v) with a 32-valued ones row at index 64 (lhsT padded to
  M=128; rows 65..127 are never read downstream).
  attnF = fp8(32*attn_out); proj PSUM = 1024*proj.
  zgF = fp8(LN1 out); FFN1 PSUM = 32*h; hts = fp8(gelu(h));
  FFN2 PSUM = 32*(ffn2 + zg) via an extra 32*identity bf16 matmul.

RoPE uses a head-dim-permuted weight layout: output feature blocks are
(y-dims of 4 heads | x-dims of 4 heads) so the per-head scores lhsT/rhs are
[32, 2, *] DoubleRow tiles with partitions h*32..h*32+32.

LayerNorm rstd = exp(-0.5*ln(var+eps)) — Ln and Exp share one ACT table
with the attention exps, so only Gelu causes table reloads.
"""

import numpy as np

B, N, C = 4, 2304, 256
NH, DH = 4, 64
HALF = DH // 2         # 32 (rope half)
NQ = N // 2            # 1152
F = 4 * C              # 1024
NCORES = 8
MT = N // 128          # 18 key tiles
MG2 = MT // 2          # 9 key-tile pairs
EPS = 1e-5
LOG2E8 = 11.541560327111707   # 8*log2(e)
EXPBIAS = 80.0                # (7+3)*8: fp8 byte bias for 8*exp(s)
JW = [(0, 512), (512, 512), (1024, 128)]
NJ = 3
EXP_DVE_G = (4,)       # key-tile-pair indices whose exp runs on DVE

_CACHE = {}


def _build_program(with_b2, debug=False):
    import concourse.tile as tile
    from concourse import bacc, mybir
    from concourse.masks import make_identity

    f32 = mybir.dt.float32
    f32r = mybir.dt.float32r
    bf16 = mybir.dt.bfloat16
    fp8 = mybir.dt.float8e4
    u8 = mybir.dt.uint8
    Alu = mybir.AluOpType
    Act = mybir.ActivationFunctionType
    DR = mybir.MatmulPerfMode.DoubleRow

    nc = bacc.Bacc(None, target_bir_lowering=False, debug=False)

    def dram(name, shape, dt=f32, out=False):
        return nc.dram_tensor(
            name, list(shape), dt, kind="ExternalOutput" if out else "ExternalInput"
        )

    d_xT = dram("xT", [C, N])
    d_xF = dram("xF", [128, 2, N], fp8)
    d_ct = dram("ct", [128, 2, N], bf16)
    d_st = dram("st", [128, 2, N], bf16)
    d_wq = dram("wq", [128, 2, 256], fp8)
    d_wqs = dram("wqs", [128, 2, 256], fp8)
    d_wk = dram("wk", [128, 2, 256], fp8)
    d_wks = dram("wks", [128, 2, 256], fp8)
    d_wv = dram("wv", [128, 2, 256], fp8)
    d_wp = dram("wp", [128, 2, 256], fp8)
    d_w1 = dram("w1", [128, 2, F], fp8)
    d_w2 = dram("w2", [128, 4, 2, 256], fp8)
    d_g1 = dram("g1c", [C, 1])
    d_g2 = dram("g2c", [C, 1])
    d_B2 = dram("B2c", [C, 1])
    d_b2 = dram("b2c", [C, 1])
    d_bf1 = dram("bf1c", [F, 1])
    d_out = dram("out", [NQ, C], out=True)
    if debug:
        d_dbg = {
            "dbg_qrot0": dram("dbg_qrot0", [64, 2, NQ], fp8, out=True),
            "dbg_qrot1": dram("dbg_qrot1", [64, 2, NQ], fp8, out=True),
            "dbg_krot0": dram("dbg_krot0", [64, 2, N], fp8, out=True),
            "dbg_krot1": dram("dbg_krot1", [64, 2, N], fp8, out=True),
            "dbg_v": dram("dbg_v", [128, MG2, 2, NH, 128], fp8, out=True),
            "dbg_attnF": dram("dbg_attnF", [128, 2, NQ], fp8, out=True),
            "dbg_zgF": dram("dbg_zgF", [128, 2, NQ], fp8, out=True),
        }

    mm = nc.tensor.matmul

    with tile.TileContext(nc) as tc:
        with tc.tile_pool(name="persist", bufs=1) as P:
            xT = [P.tile([128, N], f32, name=f"xT{i}") for i in range(2)]
            xF = P.tile([128, 2, N], fp8, name="xF")
            ct = P.tile([128, 2, N], bf16, name="ct")
            st = P.tile([128, 2, N], bf16, name="st")
            wq = P.tile([128, 2, 256], fp8, name="wq")
            wqs = P.tile([128, 2, 256], fp8, name="wqs")
            wk = P.tile([128, 2, 256], fp8, name="wk")
            wks = P.tile([128, 2, 256], fp8, name="wks")
            wv = P.tile([128, 2, 256], fp8, name="wv")
            wp = P.tile([128, 2, 256], fp8, name="wp")
            w1 = P.tile([128, 2, F], fp8, name="w1")
            w2 = P.tile([128, 4, 2, 256], fp8, name="w2")
            g1c = [P.tile([128, 1], f32, name=f"g1c{i}") for i in range(2)]
            g2c = [P.tile([128, 1], f32, name=f"g2c{i}") for i in range(2)]
            B2c = [P.tile([128, 1], f32, name=f"B2c{i}") for i in range(2)]
            b2c = [P.tile([128, 1], f32, name=f"b2c{i}") for i in range(2)]
            bf1c = P.tile([128, 8], f32, name="bf1c")
            ln8t = P.tile([128, 1], f32, name="ln8t")
            zerot = P.tile([128, 1], f32, name="zerot")
            epst = P.tile([128, 1], f32, name="epst")
            ident = P.tile([128, 128], f32, name="ident")
            identb32 = P.tile([128, 128], bf16, name="identb32")
            ones = P.tile([128, 1], f32r, name="ones")
            qrotF = [P.tile([64, 2, NQ], fp8, name=f"qrotF{i}") for i in range(2)]
            krotF = [P.tile([64, 2, N], fp8, name=f"krotF{i}") for i in range(2)]
            v_allF = P.tile([128, MG2, 2, NH, 128], fp8, name="v_allF")
            attnF = P.tile([128, 2, NQ], fp8, name="attnF")
            zgF = P.tile([128, 2, NQ], fp8, name="zgF")

            # critical-path loads first
            nc.sync.dma_start(xF, d_xF[:, :, :])
            nc.sync.dma_start(ct, d_ct[:, :, :])
            nc.sync.dma_start(st, d_st[:, :, :])
            for t, d in [(wq, d_wq), (wqs, d_wqs), (wk, d_wk), (wks, d_wks),
                         (wv, d_wv)]:
                nc.sync.dma_start(t, d[:, :, :])
            nc.sync.dma_start(wp, d_wp[:, :, :])
            nc.sync.dma_start(w1, d_w1[:, :, :])
            nc.sync.dma_start(w2, d_w2[:, :, :, :])
            for i in range(2):
                nc.sync.dma_start(g1c[i], d_g1[i * 128 : (i + 1) * 128, :])
                nc.sync.dma_start(g2c[i], d_g2[i * 128 : (i + 1) * 128, :])
                nc.sync.dma_start(B2c[i], d_B2[i * 128 : (i + 1) * 128, :])
                nc.sync.dma_start(b2c[i], d_b2[i * 128 : (i + 1) * 128, :])
                nc.sync.dma_start(xT[i], d_xT[i * 128 : (i + 1) * 128, :])
            for i in range(8):
                nc.sync.dma_start(bf1c[:, i : i + 1], d_bf1[i * 128 : (i + 1) * 128, :])

            onesf = P.tile([128, 1], f32, name="onesf")
            nc.vector.memset(onesf, 1.0)
            nc.vector.tensor_copy(ones, onesf)
            nc.vector.memset(ln8t, float(np.log(8.0)))
            nc.vector.memset(zerot, 0.0)
            nc.vector.memset(epst, EPS)
            make_identity(nc, ident)
            nc.vector.tensor_scalar_mul(identb32, ident, 32.0)
            # ones row of v lhsT (value 32); zero the M-padding rows 65..127
            nc.vector.memset(v_allF[:, :, :, :, DH : DH + 1], 32.0)
            nc.vector.memset(v_allF[:, :, :, :, DH + 1 : 128], 0.0)

            with (
                tc.tile_pool(name="psCD", bufs=1, space="PSUM") as pP,
                tc.tile_pool(name="sbCD", bufs=1) as sD,
            ):
                # ---------- phase A: q/k rope projections + v ----------
                def rope_proj(dstF, w_pre, w_shf, chunks, tag):
                    for s in range(2):
                        for (ofs, W_) in chunks:
                            sl = slice(ofs, ofs + W_)
                            pre = pP.tile([128, 512], f32, tag="sc", bufs=2,
                                          name=f"pre{tag}{s}{ofs}")
                            mm(pre[:, 0:W_], w_pre[:, :, s * 128 : (s + 1) * 128],
                               xF[:, :, sl], start=True, stop=True, perf_mode=DR)
                            shf = pP.tile([128, 512], f32, tag="sc", bufs=2,
                                          name=f"shf{tag}{s}{ofs}")
                            mm(shf[:, 0:W_], w_shf[:, :, s * 128 : (s + 1) * 128],
                               xF[:, :, sl], start=True, stop=True, perf_mode=DR)
                            t1 = sD.tile([128, 512], f32, tag="t1", bufs=2, name="t1")
                            t2 = sD.tile([128, 512], f32, tag="t2", bufs=2, name="t2")
                            nc.vector.tensor_mul(t1[:, 0:W_], pre[:, 0:W_], ct[:, s, sl])
                            nc.vector.tensor_mul(t2[:, 0:W_], shf[:, 0:W_], st[:, s, sl])
                            for hc in range(2):
                                hsl = slice(hc * 64, hc * 64 + 64)
                                nc.gpsimd.tensor_add(dstF[hc][:, s, sl],
                                                     t1[hsl, 0:W_], t2[hsl, 0:W_])

                QCH = [(0, 512), (512, 512), (1024, 128)]
                KCH = [(o, min(512, N - o)) for o in range(0, N, 512)]
                rope_proj(qrotF, wq, wqs, QCH[:1], "q0")
                rope_proj(krotF, wk, wks, KCH, "k")
                for m in range(MT):
                    g, sv = divmod(m, 2)
                    psv = pP.tile([128, 256], f32, tag="at", bufs=2, name=f"psv{m}")
                    mm(psv, xF[:, :, m * 128 : (m + 1) * 128], wv,
                       start=True, stop=True, perf_mode=DR)
                    nc.vector.tensor_copy(
                        v_allF[:, g, sv, :, 0:DH],
                        psv.rearrange("p (h d) -> p h d", h=NH),
                    )
                rope_proj(qrotF, wq, wqs, QCH[1:], "q1")

                # ---------- per-j: attention, proj+LN1, FFN, LN2, out ----------
                def attn_head(j, h):
                    ofs, W_ = JW[j]
                    jsl = slice(ofs, ofs + W_)
                    hc, hp = h // 2, slice((h % 2) * 32, (h % 2) * 32 + 32)
                    at = pP.tile([128, 512], f32, tag="at", bufs=2, name=f"at{j}{h}")
                    for g in range(MG2):
                        scp = pP.tile([128, 2, 512], f32, tag="sc", bufs=2,
                                      name=f"sc{j}{h}{g}")
                        for si in range(2):
                            m = 2 * g + si
                            mm(scp[:, si, 0:W_],
                               krotF[hc][hp, :, m * 128 : (m + 1) * 128],
                               qrotF[hc][hp, :, jsl], start=True, stop=True,
                               perf_mode=DR)
                        exf = sD.tile([128, 2, 512], fp8, tag="ex", bufs=6,
                                      name=f"ex{j}{h}{g}")
                        if g in EXP_DVE_G:
                            nc.vector.tensor_scalar(
                                exf.bitcast(u8)[:, :, 0:W_], scp[:, :, 0:W_],
                                LOG2E8 / 8192.0, EXPBIAS, Alu.mult, Alu.add)
                        else:
                            nc.scalar.activation(exf[:, :, 0:W_], scp[:, :, 0:W_],
                                                 Act.Exp, scale=1.0 / 8192.0,
                                                 bias=ln8t[:, :])
                        mm(at[:, 0:W_], v_allF[:, g, :, h, :], exf[:, :, 0:W_],
                           start=(g == 0), stop=(g == MG2 - 1), perf_mode=DR)
                    denr = sD.tile([1, 512], f32, tag="row", bufs=6,
                                   name=f"denr{j}{h}")
                    nc.vector.tensor_copy(denr[:, 0:W_], at[DH : DH + 1, 0:W_])
                    den = sD.tile([1, 512], f32, tag="row", bufs=6, name=f"den{j}{h}")
                    nc.vector.reciprocal_approx_fast(den[:, 0:W_], denr[:, 0:W_])
                    rb = sD.tile([64, 512], f32, tag="rb", bufs=2, name=f"rb{j}{h}")
                    nc.gpsimd.partition_broadcast(rb[:, 0:W_], den[0:1, 0:W_])
                    nc.vector.scalar_tensor_tensor(
                        attnF[(h % 2) * 64 : (h % 2) * 64 + 64, h // 2, jsl],
                        at[0:DH, 0:W_], 32.0, rb[:, 0:W_], Alu.mult, Alu.mult)

                def ln_rows(pair, W_, tagp):
                    """-> (sum_b, rs): raw column-sum broadcast and 1/std bcast."""
                    pssum = pP.tile([1, 512], f32, tag="at", bufs=2,
                                    name=f"pssum{tagp}")
                    for co in range(2):
                        mm(pssum[:, 0:W_], ones, pair[co],
                           start=(co == 0), stop=(co == 1))
                    pssq = pP.tile([1, 512], f32, tag="at", bufs=2,
                                   name=f"pssq{tagp}")
                    for co in range(2):
                        sq = sD.tile([128, 512], f32r, tag="sq", bufs=2,
                                     name=f"sq{tagp}{co}")
                        nc.vector.tensor_mul(sq[:, 0:W_], pair[co], pair[co])
                        mm(pssq[:, 0:W_], ones, sq[:, 0:W_],
                           start=(co == 0), stop=(co == 1))
                    sumr = sD.tile([1, 512], f32, tag="row", bufs=6,
                                   name=f"sumr{tagp}")
                    nc.vector.tensor_copy(sumr[:, 0:W_], pssum[:, 0:W_])
                    u = sD.tile([1, 512], f32, tag="row", bufs=6, name=f"u{tagp}")
                    nc.vector.scalar_tensor_tensor(u[:, 0:W_], sumr[:, 0:W_], 1.0 / C,
                                                   sumr[:, 0:W_], Alu.mult, Alu.mult)
                    w_ = sD.tile([1, 512], f32, tag="row", bufs=6, name=f"w{tagp}")
                    nc.vector.tensor_sub(w_[:, 0:W_], pssq[:, 0:W_], u[:, 0:W_])
                    # rstd = exp(-0.5*ln(var+eps)): Ln+Exp share one ACT table
                    lnv = sD.tile([1, 512], f32, tag="row", bufs=6, name=f"lnv{tagp}")
                    nc.scalar.activation(lnv[:, 0:W_], w_[:, 0:W_], Act.Ln,
                                         bias=epst[0:1, :], scale=1.0 / C)
                    rstd = sD.tile([1, 512], f32, tag="row", bufs=6,
                                   name=f"rstd{tagp}")
                    nc.scalar.activation(rstd[:, 0:W_], lnv[:, 0:W_], Act.Exp,
                                         bias=zerot[0:1, :], scale=-0.5)
                    sum_b = sD.tile([128, 512], f32, tag="bc", bufs=4,
                                    name=f"sumb{tagp}")
                    nc.gpsimd.partition_broadcast(sum_b[:, 0:W_], sumr[0:1, 0:W_])
                    rs = sD.tile([128, 512], f32, tag="bc", bufs=4, name=f"rs{tagp}")
                    nc.gpsimd.partition_broadcast(rs[:, 0:W_], rstd[0:1, 0:W_])
                    return sum_b, rs

                def d_proj_ln1(j):
                    ofs, W_ = JW[j]
                    jsl = slice(ofs, ofs + W_)
                    res = []
                    for co in range(2):
                        psp = pP.tile([128, 512], f32, tag="sc", bufs=2,
                                      name=f"psp{j}{co}")
                        mm(psp[:, 0:W_], wp[:, :, co * 128 : (co + 1) * 128],
                           attnF[:, :, jsl], start=True, stop=True, perf_mode=DR)
                        rt = sD.tile([128, 512], f32r, tag="res", bufs=4,
                                     name=f"res{j}{co}")
                        nc.vector.scalar_tensor_tensor(rt[:, 0:W_], psp[:, 0:W_],
                                                       1.0 / 1024.0, xT[co][:, jsl],
                                                       Alu.mult, Alu.add)
                        res.append(rt[:, 0:W_])
                    sum_b, rs = ln_rows(res, W_, f"a{j}")
                    zgs = []
                    for co in range(2):
                        A = sD.tile([128, 512], f32, tag="za", bufs=2,
                                    name=f"A{j}{co}")
                        nc.vector.scalar_tensor_tensor(A[:, 0:W_], sum_b[:, 0:W_],
                                                       -1.0 / C, res[co],
                                                       Alu.mult, Alu.add)
                        z = sD.tile([128, 512], bf16, tag="zg", bufs=4,
                                    name=f"zg{j}{co}")
                        nc.vector.scalar_tensor_tensor(z[:, 0:W_], A[:, 0:W_],
                                                       g1c[co], rs[:, 0:W_],
                                                       Alu.mult, Alu.mult)
                        nc.gpsimd.tensor_copy(zgF[:, co, jsl], z[:, 0:W_])
                        zgs.append(z[:, 0:W_])
                    return zgs

                def d_ffn1(j):
                    ofs, W_ = JW[j]
                    jsl = slice(ofs, ofs + W_)
                    hts = sD.tile([128, 8, 512], fp8, tag="hts", bufs=2,
                                  name=f"hts{j}")
                    for f in range(8):
                        psh = pP.tile([128, 512], f32, tag="sc", bufs=2,
                                      name=f"psh{j}{f}")
                        mm(psh[:, 0:W_], w1[:, :, f * 128 : (f + 1) * 128],
                           zgF[:, :, jsl], start=True, stop=True, perf_mode=DR)
                        nc.scalar.activation(hts[:, f, 0:W_], psh[:, 0:W_], Act.Gelu,
                                             bias=bf1c[:, f : f + 1], scale=1.0 / 32.0)
                    return hts

                def d_ffn2_ln2_out(j, zgs, hts):
                    ofs, W_ = JW[j]
                    x2l = []
                    for co in range(2):
                        psf = pP.tile([128, 512], f32, tag="sc", bufs=2,
                                      name=f"psf{j}{co}")
                        for g2 in range(4):
                            mm(psf[:, 0:W_], w2[:, g2, :, co * 128 : (co + 1) * 128],
                               hts[:, 2 * g2 : 2 * g2 + 2, 0:W_],
                               start=(g2 == 0), stop=False, perf_mode=DR)
                        mm(psf[:, 0:W_], identb32, zgs[co],
                           start=False, stop=True)
                        x2 = sD.tile([128, 512], f32r, tag="x2", bufs=4,
                                     name=f"x2_{j}{co}")
                        nc.vector.tensor_scalar(x2[:, 0:W_], psf[:, 0:W_],
                                                1.0 / 32.0, B2c[co],
                                                Alu.mult, Alu.add)
                        x2l.append(x2[:, 0:W_])
                    sum_b2, rs2 = ln_rows(x2l, W_, f"b{j}")
                    for co in range(2):
                        A2 = sD.tile([128, 512], f32, tag="za", bufs=2,
                                     name=f"A2_{j}{co}")
                        nc.vector.scalar_tensor_tensor(A2[:, 0:W_], sum_b2[:, 0:W_],
                                                       -1.0 / C, x2l[co],
                                                       Alu.mult, Alu.add)
                        fz = sD.tile([128, 512], f32, tag="fz", bufs=2,
                                     name=f"fz{j}{co}")
                        nc.vector.scalar_tensor_tensor(fz[:, 0:W_], A2[:, 0:W_],
                                                       g2c[co], rs2[:, 0:W_],
                                                       Alu.mult, Alu.mult)
                        if with_b2:
                            fo = sD.tile([128, 512], f32, tag="fz", bufs=2,
                                         name=f"fo{j}{co}")
                            nc.gpsimd.tensor_scalar_add(fo[:, 0:W_], fz[:, 0:W_],
                                                        b2c[co])
                            fz = fo
                        x2l[co] = fz
                    for tt in range(W_ // 128):
                        tok = sD.tile([128, C], f32, tag="tok", bufs=3,
                                      name=f"tok{j}{tt}")
                        for co in range(2):
                            pst = pP.tile([128, 128], f32, tag="at", bufs=2,
                                          name=f"pst{j}{tt}{co}")
                            nc.tensor.transpose(
                                pst, x2l[co][:, tt * 128 : (tt + 1) * 128], ident
                            )
                            nc.vector.tensor_copy(
                                tok[:, co * 128 : (co + 1) * 128], pst
                            )
                        t0 = ofs + tt * 128
                        nc.sync.dma_start(d_out[t0 : t0 + 128, :], tok)

                for j in range(NJ):
                    for h in range(NH):
                        attn_head(j, h)
                    zgs = d_proj_ln1(j)
                    hts = d_ffn1(j)
                    d_ffn2_ln2_out(j, zgs, hts)

                if debug:
                    for i in range(2):
                        nc.sync.dma_start(d_dbg["dbg_qrot%d" % i][:, :, :], qrotF[i])
                        nc.sync.dma_start(d_dbg["dbg_krot%d" % i][:, :, :], krotF[i])
                    nc.sync.dma_start(d_dbg["dbg_v"][:, :, :, :, :], v_allF)
                    nc.sync.dma_start(d_dbg["dbg_attnF"][:, :, :], attnF)
                    nc.sync.dma_start(d_dbg["dbg_zgF"][:, :, :], zgF)

    nc.compile()
    return nc


def _get_program(with_b2, debug=False):
    key = f"nc{int(with_b2)}_{int(debug)}"
    if key not in _CACHE:
        _CACHE[key] = _build_program(with_b2, debug)
    return _CACHE[key]


def _host_prep(x, Wqkv, Wproj, g1, b1, g2, b2, W1, bf1, W2, bf2, H, W):
    import ml_dtypes

    f8 = ml_dtypes.float8_e4m3
    bf = ml_dtypes.bfloat16
    f32 = np.float32

    Wq, Wk, Wv = Wqkv[0:C], Wqkv[C : 2 * C], Wqkv[2 * C : 3 * C]

    # feature orders: block s'=0 -> y-dims (h*64+d, d<32), s'=1 -> x-dims
    featY = np.concatenate([np.arange(HALF) + h * DH for h in range(NH)])
    featX = featY + HALF
    pair = lambda ff: (ff ^ 1)  # rotate-half partner within a rope half

    def proj_tiles(Wm):
        """-> (pre [128,2,256], shf [128,2,256]) fp8, x32, out cols = [Y|X]."""
        cols = np.concatenate([featY, featX])
        colss = np.concatenate([pair(featY), pair(featX)])
        pre = (Wm[cols] * 32.0).T.reshape(2, 128, 256).transpose(1, 0, 2)
        shf = (Wm[colss] * 32.0).T.reshape(2, 128, 256).transpose(1, 0, 2)
        return (np.ascontiguousarray(pre).astype(f8),
                np.ascontiguousarray(shf).astype(f8))

    wq8, wqs8 = proj_tiles(Wq)
    wk8, wks8 = proj_tiles(Wk)
    wv8 = np.ascontiguousarray((Wv * 32.0).T.reshape(2, 128, 256)
                               .transpose(1, 0, 2)).astype(f8)
    wp8 = np.ascontiguousarray((Wproj * 32.0).T.reshape(2, 128, 256)
                               .transpose(1, 0, 2)).astype(f8)
    w18 = np.ascontiguousarray((W1 * 32.0).T.reshape(2, 128, F)
                               .transpose(1, 0, 2)).astype(f8)
    # w2 [128, 4, 2, 256]: in-feature (g2, s, p) = (2*g2+s)*128+p
    w28 = np.ascontiguousarray((W2 * 32.0).T.reshape(4, 2, 128, C)
                               .transpose(2, 0, 1, 3)).astype(f8)

    shared = {
        "wq": wq8, "wqs": wqs8, "wk": wk8, "wks": wks8, "wv": wv8, "wp": wp8,
        "w1": w18, "w2": w28,
        "g1c": np.ascontiguousarray(g1.reshape(C, 1), dtype=f32),
        "g2c": np.ascontiguousarray(g2.reshape(C, 1), dtype=f32),
        "B2c": np.ascontiguousarray((b1 + bf2).reshape(C, 1), dtype=f32),
        "b2c": np.ascontiguousarray(b2.reshape(C, 1), dtype=f32),
        "bf1c": np.ascontiguousarray((bf1 + W1 @ b1).reshape(F, 1), dtype=f32),
    }

    # rope tables in the permuted row layout: row h*32+dp, s in {y, x}
    invf = 1.0 / (10000.0 ** (np.arange(HALF, dtype=np.float64) / HALF))
    yy, xx = np.meshgrid(np.arange(H), np.arange(W), indexing="ij")
    pos_y = yy.reshape(-1).astype(np.float64)
    pos_x = xx.reshape(-1).astype(np.float64)
    ay = invf[:, None] * pos_y[None, :]   # [32, N]
    ax = invf[:, None] * pos_x[None, :]
    sgn = np.where(np.arange(HALF) % 2 == 0, -1.0, 1.0)[:, None]
    ct32 = np.stack([np.cos(ay), np.cos(ax)], axis=1)          # [32, 2, N]
    st32 = np.stack([np.sin(ay) * sgn, np.sin(ax) * sgn], axis=1)
    ct128 = np.tile(ct32, (NH, 1, 1))                           # [128, 2, N]
    st128 = np.tile(st32, (NH, 1, 1))

    in_maps = []
    for core in range(NCORES):
        b, qh = core // 2, core % 2
        n0 = qh * NQ
        rot = np.concatenate([np.arange(n0, N), np.arange(0, n0)])
        m = dict(shared)
        xb = x[b][rot]                                          # [N, C]
        m["xT"] = np.ascontiguousarray(xb.T, dtype=f32)
        m["xF"] = np.ascontiguousarray(
            xb.T.reshape(2, 128, N).transpose(1, 0, 2)).astype(f8)
        m["ct"] = np.ascontiguousarray(ct128[:, :, rot]).astype(bf)
        m["st"] = np.ascontiguousarray(st128[:, :, rot]).astype(bf)
        in_maps.append(m)
    return in_maps


def kernel(x, Wqkv, Wproj, g1, b1, g2, b2, W1, bf1, W2, bf2, H, W, **kw):
    from concourse.bass_utils import run_bass_kernel_spmd

    x = np.asarray(x, dtype=np.float32)
    args = [np.asarray(a, dtype=np.float32)
            for a in (Wqkv, Wproj, g1, b1, g2, b2, W1, bf1, W2, bf2)]
    H, W = int(H), int(W)

    with_b2 = bool(np.any(args[5]))
    nc = _get_program(with_b2, _CACHE.get("debug", False))
    in_maps = _host_prep(x, *args, H, W)
    res = run_bass_kernel_spmd(nc, in_maps, core_ids=list(range(NCORES)),
                               **_CACHE.get("run_kwargs", {}))
    _CACHE["last_result"] = res

    out = np.zeros((B, N, C), dtype=np.float32)
    for core in range(NCORES):
        b, qh = core // 2, core % 2
        n0 = qh * NQ
        out[b, n0 : n0 + NQ, :] = res.results[core]["out"]
    return out


# revision 9
# speedup vs baseline: 1.1304x; 1.1304x over previous
"""Self-contained Trainium2 Bass kernel for the AttnBlock problem — fp8 edition.

Sharding: 8 cores; core c handles batch b = c//2, query rows
[qh*1152, (qh+1)*1152) with qh = c%2.  Each core computes full K/V for its
batch (duplicated across the 2 cores of a batch) so there are NO collectives.
Per-core token order is rotated so each core's own query tokens are always
columns 0..1151.

All heavy matmuls run in fp8e4m3 with DoubleRow perf mode (2 contraction
subtiles per pass, 0.5 cycles/column).  Weights are host-scaled by 32 into
the fp8 sweet spot; activations carry powers-of-two scales that are folded
into downstream scalars:
  qrot/krot = bf16(32*rope(q|k)) (scores stay bf16 — contraction is 64, so
  fp8 wins nothing there), scores PSUM = 8192*s (s = q.k/sqrt(64))
  exp: 8*exp(s) as fp8 — on ACT (Exp table, scale 1/8192, bias ln8) or on
  DVE via the direct fp8-byte trick u8 = round(8*log2(e)*s + 80).
  v_allF = fp8(32*v) with a 32-valued ones row at index 64 (lhsT padded to
  M=128; rows 65..127 are never read downstream).
  attnF = fp8(32*attn_out); proj PSUM = 1024*proj.
  zgF = fp8(LN1 out); FFN1 PSUM = 32*h; hts = fp8(gelu(h));
  FFN2 PSUM = 32*(ffn2 + zg) via an extra 32*identity bf16 matmul.

LayerNorm rstd = exp(-0.5*ln(var+eps)) — Ln and Exp share one ACT table
with the attention exps, so only Gelu causes table reloads.
"""

import numpy as np

B, N, C = 4, 2304, 256
NH, DH = 4, 64
HALF = DH // 2         # 32 (rope half)
NQ = N // 2            # 1152
F = 4 * C              # 1024
NCORES = 8
MT = N // 128          # 18 key tiles
MG2 = MT // 2          # 9 key-tile pairs
EPS = 1e-5
LOG2E8 = 11.541560327111707   # 8*log2(e)
EXPBIAS = 80.0                # (7+3)*8: fp8 byte bias for 8*exp(s)
JW = [(0, 512), (512, 512), (1024, 128)]
NJ = 3
EXP_DVE_G = (4,)       # key-tile-pair indices whose exp runs on DVE

_CACHE = {}


def _build_program(with_b2, debug=False):
    import concourse.tile as tile
    from concourse import bacc, mybir
    from concourse.masks import make_identity

    f32 = mybir.dt.float32
    f32r = mybir.dt.float32r
    bf16 = mybir.dt.bfloat16
    fp8 = mybir.dt.float8e4
    u8 = mybir.dt.uint8
    Alu = mybir.AluOpType
    Act = mybir.ActivationFunctionType
    DR = mybir.MatmulPerfMode.DoubleRow

    nc = bacc.Bacc(None, target_bir_lowering=False, debug=False)

    def dram(name, shape, dt=f32, out=False):
        return nc.dram_tensor(
            name, list(shape), dt, kind="ExternalOutput" if out else "ExternalInput"
        )

    d_xT = dram("xT", [C, N])
    d_xF = dram("xF", [128, 2, N], fp8)
    d_ct = dram("ct", [128, N], bf16)
    d_st = dram("st", [128, N], bf16)
    d_wq = dram("wq", [128, 2, 256], fp8)
    d_wqs = dram("wqs", [128, 2, 256], fp8)
    d_wk = dram("wk", [128, 2, 256], fp8)
    d_wks = dram("wks", [128, 2, 256], fp8)
    d_wv = dram("wv", [128, 2, 256], fp8)
    d_wp = dram("wp", [128, 2, 256], fp8)
    d_w1 = dram("w1", [128, 2, F], fp8)
    d_w2 = dram("w2", [128, 4, 2, 256], fp8)
    d_g1 = dram("g1c", [C, 1])
    d_g2 = dram("g2c", [C, 1])
    d_B2 = dram("B2c", [C, 1])
    d_b2 = dram("b2c", [C, 1])
    d_bf1 = dram("bf1c", [F, 1])
    d_out = nc.dram_tensor("out", [NQ, C], bf16, kind="ExternalOutput")
    if debug:
        d_dbg = {
            "dbg_qrot0": nc.dram_tensor("dbg_qrot0", [128, NQ], bf16, kind="ExternalOutput"),
            "dbg_qrot1": nc.dram_tensor("dbg_qrot1", [128, NQ], bf16, kind="ExternalOutput"),
            "dbg_krot0": nc.dram_tensor("dbg_krot0", [128, N], bf16, kind="ExternalOutput"),
            "dbg_krot1": nc.dram_tensor("dbg_krot1", [128, N], bf16, kind="ExternalOutput"),
            "dbg_v": dram("dbg_v", [128, MG2, 2, NH, 128], fp8, out=True),
            "dbg_attnF": dram("dbg_attnF", [128, 2, NQ], fp8, out=True),
            "dbg_zgF": dram("dbg_zgF", [128, 2, NQ], fp8, out=True),
        }

    mm = nc.tensor.matmul

    with tile.TileContext(nc) as tc:
        with tc.tile_pool(name="persist", bufs=1) as P:
            xT = [P.tile([128, N], f32, name=f"xT{i}") for i in range(2)]
            xF = P.tile([128, 2, N], fp8, name="xF")
            ct = P.tile([128, N], bf16, name="ct")
            st = P.tile([128, N], bf16, name="st")
            wq = P.tile([128, 2, 256], fp8, name="wq")
            wqs = P.tile([128, 2, 256], fp8, name="wqs")
            wk = P.tile([128, 2, 256], fp8, name="wk")
            wks = P.tile([128, 2, 256], fp8, name="wks")
            wv = P.tile([128, 2, 256], fp8, name="wv")
            wp = P.tile([128, 2, 256], fp8, name="wp")
            w1 = P.tile([128, 2, F], fp8, name="w1")
            w2 = P.tile([128, 4, 2, 256], fp8, name="w2")
            g1c = [P.tile([128, 1], f32, name=f"g1c{i}") for i in range(2)]
            g2c = [P.tile([128, 1], f32, name=f"g2c{i}") for i in range(2)]
            B2c = [P.tile([128, 1], f32, name=f"B2c{i}") for i in range(2)]
            b2c = [P.tile([128, 1], f32, name=f"b2c{i}") for i in range(2)]
            bf1c = P.tile([128, 8], f32, name="bf1c")
            ln8t = P.tile([128, 1], f32, name="ln8t")
            zerot = P.tile([128, 1], f32, name="zerot")
            epst = P.tile([128, 1], f32, name="epst")
            identb = P.tile([128, 128], bf16, name="identb")
            ones = P.tile([128, 1], f32r, name="ones")
            qrot = [P.tile([128, NQ], bf16, name=f"qrot{i}") for i in range(2)]
            krot = [P.tile([128, N], bf16, name=f"krot{i}") for i in range(2)]
            v_allF = P.tile([128, MG2, 2, NH, 128], fp8, name="v_allF")
            attnF = P.tile([128, 2, NQ], fp8, name="attnF")
            zgF = P.tile([128, 2, NQ], fp8, name="zgF")

            # critical-path loads first
            nc.sync.dma_start(xF, d_xF[:, :, :])
            nc.sync.dma_start(ct, d_ct[:, :])
            nc.sync.dma_start(st, d_st[:, :])
            for t, d in [(wq, d_wq), (wqs, d_wqs), (wk, d_wk), (wks, d_wks),
                         (wv, d_wv)]:
                nc.sync.dma_start(t, d[:, :, :])
            nc.sync.dma_start(wp, d_wp[:, :, :])
            nc.sync.dma_start(w1, d_w1[:, :, :])
            nc.sync.dma_start(w2, d_w2[:, :, :, :])
            for i in range(2):
                nc.sync.dma_start(g1c[i], d_g1[i * 128 : (i + 1) * 128, :])
                nc.sync.dma_start(g2c[i], d_g2[i * 128 : (i + 1) * 128, :])
                nc.sync.dma_start(B2c[i], d_B2[i * 128 : (i + 1) * 128, :])
                nc.sync.dma_start(b2c[i], d_b2[i * 128 : (i + 1) * 128, :])
                nc.sync.dma_start(xT[i], d_xT[i * 128 : (i + 1) * 128, :])
            for i in range(8):
                nc.sync.dma_start(bf1c[:, i : i + 1], d_bf1[i * 128 : (i + 1) * 128, :])

            onesf = P.tile([128, 1], f32, name="onesf")
            nc.vector.memset(onesf, 1.0)
            nc.vector.tensor_copy(ones, onesf)
            nc.vector.memset(ln8t, float(np.log(8.0)))
            nc.vector.memset(zerot, 0.0)
            nc.vector.memset(epst, EPS)
            identf = P.tile([128, 128], f32, name="identf")
            make_identity(nc, identf)
            nc.vector.tensor_copy(identb, identf)
            # ones row of v lhsT (value 32); zero the M-padding rows 65..127
            nc.vector.memset(v_allF[:, :, :, :, DH : DH + 1], 32.0)
            nc.vector.memset(v_allF[:, :, :, :, DH + 1 : 128], 0.0)

            with (
                tc.tile_pool(name="psCD", bufs=1, space="PSUM") as pP,
                tc.tile_pool(name="sbCD", bufs=1) as sD,
            ):
                # ---------- phase A: q/k rope projections + v ----------
                def rope_proj(dst, w_pre, w_shf, chunks, tag):
                    for cc in range(2):
                        for (ofs, W_) in chunks:
                            sl = slice(ofs, ofs + W_)
                            pre = pP.tile([128, 512], f32, tag="sc", bufs=2,
                                          name=f"pre{tag}{cc}{ofs}")
                            mm(pre[:, 0:W_], w_pre[:, :, cc * 128 : (cc + 1) * 128],
                               xF[:, :, sl], start=True, stop=True, perf_mode=DR)
                            shf = pP.tile([128, 512], f32, tag="sc", bufs=2,
                                          name=f"shf{tag}{cc}{ofs}")
                            mm(shf[:, 0:W_], w_shf[:, :, cc * 128 : (cc + 1) * 128],
                               xF[:, :, sl], start=True, stop=True, perf_mode=DR)
                            t1 = sD.tile([128, 512], f32, tag="t1", bufs=2, name="t1")
                            t2 = sD.tile([128, 512], f32, tag="t2", bufs=2, name="t2")
                            nc.vector.tensor_mul(t1[:, 0:W_], pre[:, 0:W_], ct[:, sl])
                            nc.vector.tensor_mul(t2[:, 0:W_], shf[:, 0:W_], st[:, sl])
                            nc.gpsimd.tensor_add(dst[cc][:, sl],
                                                 t1[:, 0:W_], t2[:, 0:W_])

                QCH = [(0, 512), (512, 512), (1024, 128)]
                KCH = [(o, min(512, N - o)) for o in range(0, N, 512)]
                rope_proj(qrot, wq, wqs, QCH[:1], "q0")
                rope_proj(krot, wk, wks, KCH, "k")
                for m in range(MT):
                    g, sv = divmod(m, 2)
                    psv = pP.tile([128, 256], f32, tag="at", bufs=2, name=f"psv{m}")
                    mm(psv, xF[:, :, m * 128 : (m + 1) * 128], wv,
                       start=True, stop=True, perf_mode=DR)
                    nc.vector.tensor_copy(
                        v_allF[:, g, sv, :, 0:DH],
                        psv.rearrange("p (h d) -> p h d", h=NH),
                    )
                rope_proj(qrot, wq, wqs, QCH[1:], "q1")

                # ---------- per-j: attention, proj+LN1, FFN, LN2, out ----------
                def attn_head(j, h):
                    ofs, W_ = JW[j]
                    jsl = slice(ofs, ofs + W_)
                    hc, hp = h // 2, slice((h % 2) * 64, (h % 2) * 64 + 64)
                    at = pP.tile([128, 512], f32, tag="at", bufs=2, name=f"at{j}{h}")
                    for g in range(MG2):
                        scp = pP.tile([128, 2, 512], f32, tag="sc", bufs=2,
                                      name=f"sc{j}{h}{g}")
                        for si in range(2):
                            m = 2 * g + si
                            mm(scp[:, si, 0:W_],
                               krot[hc][hp, m * 128 : (m + 1) * 128],
                               qrot[hc][hp, jsl], start=True, stop=True)
                        exf = sD.tile([128, 2, 512], fp8, tag="ex", bufs=6,
                                      name=f"ex{j}{h}{g}")
                        if g in EXP_DVE_G:
                            nc.vector.tensor_scalar(
                                exf.bitcast(u8)[:, :, 0:W_], scp[:, :, 0:W_],
                                LOG2E8 / 8192.0, EXPBIAS, Alu.mult, Alu.add)
                        else:
                            nc.scalar.activation(exf[:, :, 0:W_], scp[:, :, 0:W_],
                                                 Act.Exp, scale=1.0 / 8192.0,
                                                 bias=ln8t[:, :])
                        mm(at[:, 0:W_], v_allF[:, g, :, h, :], exf[:, :, 0:W_],
                           start=(g == 0), stop=(g == MG2 - 1), perf_mode=DR)
                    denr = sD.tile([1, 512], f32, tag="row", bufs=6,
                                   name=f"denr{j}{h}")
                    nc.vector.tensor_copy(denr[:, 0:W_], at[DH : DH + 1, 0:W_])
                    den = sD.tile([1, 512], f32, tag="row", bufs=6, name=f"den{j}{h}")
                    nc.vector.reciprocal_approx_fast(den[:, 0:W_], denr[:, 0:W_])
                    rb = sD.tile([64, 512], f32, tag="rb", bufs=2, name=f"rb{j}{h}")
                    nc.gpsimd.partition_broadcast(rb[:, 0:W_], den[0:1, 0:W_])
                    nc.vector.scalar_tensor_tensor(
                        attnF[(h % 2) * 64 : (h % 2) * 64 + 64, h // 2, jsl],
                        at[0:DH, 0:W_], 32.0, rb[:, 0:W_], Alu.mult, Alu.mult)

                def ln_rows(pair, W_, tagp):
                    """-> (sum_b, rs): raw column-sum broadcast and 1/std bcast."""
                    pssum = pP.tile([1, 512], f32, tag="at", bufs=2,
                                    name=f"pssum{tagp}")
                    for co in range(2):
                        mm(pssum[:, 0:W_], ones, pair[co],
                           start=(co == 0), stop=(co == 1))
                    pssq = pP.tile([1, 512], f32, tag="at", bufs=2,
                                   name=f"pssq{tagp}")
                    for co in range(2):
                        sq = sD.tile([128, 512], f32r, tag="sq", bufs=2,
                                     name=f"sq{tagp}{co}")
                        nc.vector.tensor_mul(sq[:, 0:W_], pair[co], pair[co])
                        mm(pssq[:, 0:W_], ones, sq[:, 0:W_],
                           start=(co == 0), stop=(co == 1))
                    sumr = sD.tile([1, 512], f32, tag="row", bufs=6,
                                   name=f"sumr{tagp}")
                    nc.vector.tensor_copy(sumr[:, 0:W_], pssum[:, 0:W_])
                    u = sD.tile([1, 512], f32, tag="row", bufs=6, name=f"u{tagp}")
                    nc.vector.scalar_tensor_tensor(u[:, 0:W_], sumr[:, 0:W_], 1.0 / C,
                                                   sumr[:, 0:W_], Alu.mult, Alu.mult)
                    w_ = sD.tile([1, 512], f32, tag="row", bufs=6, name=f"w{tagp}")
                    nc.vector.tensor_sub(w_[:, 0:W_], pssq[:, 0:W_], u[:, 0:W_])
                    # rstd = exp(-0.5*ln(var+eps)): Ln+Exp share one ACT table
                    lnv = sD.tile([1, 512], f32, tag="row", bufs=6, name=f"lnv{tagp}")
                    nc.scalar.activation(lnv[:, 0:W_], w_[:, 0:W_], Act.Ln,
                                         bias=epst[0:1, :], scale=1.0 / C)
                    rstd = sD.tile([1, 512], f32, tag="row", bufs=6,
                                   name=f"rstd{tagp}")
                    nc.scalar.activation(rstd[:, 0:W_], lnv[:, 0:W_], Act.Exp,
                                         bias=zerot[0:1, :], scale=-0.5)
                    sum_b = sD.tile([128, 512], f32, tag="bc", bufs=4,
                                    name=f"sumb{tagp}")
                    nc.gpsimd.partition_broadcast(sum_b[:, 0:W_], sumr[0:1, 0:W_])
                    rs = sD.tile([128, 512], f32, tag="bc", bufs=4, name=f"rs{tagp}")
                    nc.gpsimd.partition_broadcast(rs[:, 0:W_], rstd[0:1, 0:W_])
                    return sum_b, rs

                def d_proj_ln1(j):
                    ofs, W_ = JW[j]
                    jsl = slice(ofs, ofs + W_)
                    res = []
                    for co in range(2):
                        psp = pP.tile([128, 512], f32, tag="sc", bufs=2,
                                      name=f"psp{j}{co}")
                        mm(psp[:, 0:W_], wp[:, :, co * 128 : (co + 1) * 128],
                           attnF[:, :, jsl], start=True, stop=True, perf_mode=DR)
                        rt = sD.tile([128, 512], f32r, tag="res", bufs=4,
                                     name=f"res{j}{co}")
                        nc.vector.scalar_tensor_tensor(rt[:, 0:W_], psp[:, 0:W_],
                                                       1.0 / 1024.0, xT[co][:, jsl],
                                                       Alu.mult, Alu.add)
                        res.append(rt[:, 0:W_])
                    sum_b, rs = ln_rows(res, W_, f"a{j}")
                    zgs = []
                    for co in range(2):
                        A = sD.tile([128, 512], f32, tag="za", bufs=2,
                                    name=f"A{j}{co}")
                        nc.vector.scalar_tensor_tensor(A[:, 0:W_], sum_b[:, 0:W_],
                                                       -1.0 / C, res[co],
                                                       Alu.mult, Alu.add)
                        z = sD.tile([128, 512], bf16, tag="zg", bufs=4,
                                    name=f"zg{j}{co}")
                        nc.vector.scalar_tensor_tensor(z[:, 0:W_], A[:, 0:W_],
                                                       g1c[co], rs[:, 0:W_],
                                                       Alu.mult, Alu.mult)
                        nc.gpsimd.tensor_copy(zgF[:, co, jsl], z[:, 0:W_])
                        zgs.append(z[:, 0:W_])
                    return zgs

                def d_ffn1(j):
                    ofs, W_ = JW[j]
                    jsl = slice(ofs, ofs + W_)
                    hts = sD.tile([128, 8, 512], fp8, tag="hts", bufs=2,
                                  name=f"hts{j}")
                    for f in range(8):
                        psh = pP.tile([128, 512], f32, tag="sc", bufs=2,
                                      name=f"psh{j}{f}")
                        mm(psh[:, 0:W_], w1[:, :, f * 128 : (f + 1) * 128],
                           zgF[:, :, jsl], start=True, stop=True, perf_mode=DR)
                        nc.scalar.activation(hts[:, f, 0:W_], psh[:, 0:W_], Act.Gelu,
                                             bias=bf1c[:, f : f + 1], scale=1.0 / 32.0)
                    return hts

                def d_ffn2_ln2_out(j, zgs, hts):
                    ofs, W_ = JW[j]
                    x2l = []
                    for co in range(2):
                        psf = pP.tile([128, 512], f32, tag="sc", bufs=2,
                                      name=f"psf{j}{co}")
                        for g2 in range(4):
                            mm(psf[:, 0:W_], w2[:, g2, :, co * 128 : (co + 1) * 128],
                               hts[:, 2 * g2 : 2 * g2 + 2, 0:W_],
                               start=(g2 == 0), stop=(g2 == 3), perf_mode=DR)
                        x2t = sD.tile([128, 512], f32, tag="za", bufs=2,
                                      name=f"x2t{j}{co}")
                        nc.vector.tensor_scalar(x2t[:, 0:W_], psf[:, 0:W_],
                                                1.0 / 32.0, B2c[co],
                                                Alu.mult, Alu.add)
                        x2 = sD.tile([128, 512], f32r, tag="x2", bufs=4,
                                     name=f"x2_{j}{co}")
                        nc.vector.tensor_add(x2[:, 0:W_], x2t[:, 0:W_], zgs[co])
                        x2l.append(x2[:, 0:W_])
                    sum_b2, rs2 = ln_rows(x2l, W_, f"b{j}")
                    for co in range(2):
                        A2 = sD.tile([128, 512], f32, tag="za", bufs=2,
                                     name=f"A2_{j}{co}")
                        nc.vector.scalar_tensor_tensor(A2[:, 0:W_], sum_b2[:, 0:W_],
                                                       -1.0 / C, x2l[co],
                                                       Alu.mult, Alu.add)
                        fz = sD.tile([128, 512], bf16, tag="fz", bufs=2,
                                     name=f"fz{j}{co}")
                        nc.vector.scalar_tensor_tensor(fz[:, 0:W_], A2[:, 0:W_],
                                                       g2c[co], rs2[:, 0:W_],
                                                       Alu.mult, Alu.mult)
                        if with_b2:
                            fo = sD.tile([128, 512], bf16, tag="fz", bufs=2,
                                         name=f"fo{j}{co}")
                            nc.gpsimd.tensor_scalar_add(fo[:, 0:W_], fz[:, 0:W_],
                                                        b2c[co])
                            fz = fo
                        x2l[co] = fz
                    for tt in range(W_ // 128):
                        t0 = ofs + tt * 128
                        tok = sD.tile([128, C], bf16, tag="tok", bufs=3,
                                      name=f"tok{j}{tt}")
                        for co in range(2):
                            pst = pP.tile([128, 128], bf16, tag="at", bufs=2,
                                          name=f"pst{j}{tt}{co}")
                            nc.tensor.transpose(
                                pst, x2l[co][:, tt * 128 : (tt + 1) * 128], identb
                            )
                            csl = slice(co * 128, (co + 1) * 128)
                            if co == 0:
                                nc.vector.tensor_copy(tok[:, csl], pst)
                            else:
                                nc.scalar.copy(tok[:, csl], pst)
                        nc.sync.dma_start(d_out[t0 : t0 + 128, :], tok)

                for j in range(NJ):
                    for h in range(NH):
                        attn_head(j, h)
                    zgs = d_proj_ln1(j)
                    hts = d_ffn1(j)
                    d_ffn2_ln2_out(j, zgs, hts)

                if debug:
                    for i in range(2):
                        nc.sync.dma_start(d_dbg["dbg_qrot%d" % i][:, :], qrot[i])
                        nc.sync.dma_start(d_dbg["dbg_krot%d" % i][:, :], krot[i])
                    nc.sync.dma_start(d_dbg["dbg_v"][:, :, :, :, :], v_allF)
                    nc.sync.dma_start(d_dbg["dbg_attnF"][:, :, :], attnF)
                    nc.sync.dma_start(d_dbg["dbg_zgF"][:, :, :], zgF)

    nc.compile()
    return nc


def _get_program(with_b2, debug=False):
    key = f"nc{int(with_b2)}_{int(debug)}"
    if key not in _CACHE:
        _CACHE[key] = _build_program(with_b2, debug)
    return _CACHE[key]


def _host_prep(x, Wqkv, Wproj, g1, b1, g2, b2, W1, bf1, W2, bf2, H, W):
    import ml_dtypes

    f8 = ml_dtypes.float8_e4m3
    bf = ml_dtypes.bfloat16
    f32 = np.float32

    Wq, Wk, Wv = Wqkv[0:C], Wqkv[C : 2 * C], Wqkv[2 * C : 3 * C]

    # rotate-half partner: pair-swap within each 32-dim rope half (d ^ 1)
    permC = np.arange(C) ^ 1

    def proj_tiles(Wm):
        """-> (pre, shf) [128, 2, 256] fp8, x32, out cols in original order."""
        pre = (Wm * 32.0).T.reshape(2, 128, 256).transpose(1, 0, 2)
        shf = (Wm[permC] * 32.0).T.reshape(2, 128, 256).transpose(1, 0, 2)
        return (np.ascontiguousarray(pre).astype(f8),
                np.ascontiguousarray(shf).astype(f8))

    wq8, wqs8 = proj_tiles(Wq)
    wk8, wks8 = proj_tiles(Wk)
    wv8 = np.ascontiguousarray((Wv * 32.0).T.reshape(2, 128, 256)
                               .transpose(1, 0, 2)).astype(f8)
    wp8 = np.ascontiguousarray((Wproj * 32.0).T.reshape(2, 128, 256)
                               .transpose(1, 0, 2)).astype(f8)
    w18 = np.ascontiguousarray((W1 * 32.0).T.reshape(2, 128, F)
                               .transpose(1, 0, 2)).astype(f8)
    # w2 [128, 4, 2, 256]: in-feature (g2, s, p) = (2*g2+s)*128+p
    w28 = np.ascontiguousarray((W2 * 32.0).T.reshape(4, 2, 128, C)
                               .transpose(2, 0, 1, 3)).astype(f8)

    shared = {
        "wq": wq8, "wqs": wqs8, "wk": wk8, "wks": wks8, "wv": wv8, "wp": wp8,
        "w1": w18, "w2": w28,
        "g1c": np.ascontiguousarray(g1.reshape(C, 1), dtype=f32),
        "g2c": np.ascontiguousarray(g2.reshape(C, 1), dtype=f32),
        "B2c": np.ascontiguousarray((b1 + bf2).reshape(C, 1), dtype=f32),
        "b2c": np.ascontiguousarray(b2.reshape(C, 1), dtype=f32),
        "bf1c": np.ascontiguousarray((bf1 + W1 @ b1).reshape(F, 1), dtype=f32),
    }

    # rope tables, baseline row layout: row d (of 64-dim head block) x 2
    invf = 1.0 / (10000.0 ** (np.arange(HALF, dtype=np.float64) / HALF))
    yy, xx = np.meshgrid(np.arange(H), np.arange(W), indexing="ij")
    pos_y = yy.reshape(-1).astype(np.float64)
    pos_x = xx.reshape(-1).astype(np.float64)
    ang = np.concatenate(
        [invf[:, None] * pos_y[None, :], invf[:, None] * pos_x[None, :]], axis=0
    )  # [64, N]
    sgn = np.where(np.arange(DH) % 2 == 0, -1.0, 1.0)[:, None]
    ct64 = np.cos(ang)
    st64 = np.sin(ang) * sgn
    ct128 = np.concatenate([ct64, ct64], axis=0)  # [128, N]
    st128 = np.concatenate([st64, st64], axis=0)

    in_maps = []
    for core in range(NCORES):
        b, qh = core // 2, core % 2
        n0 = qh * NQ
        rot = np.concatenate([np.arange(n0, N), np.arange(0, n0)])
        m = dict(shared)
        xb = x[b][rot]                                          # [N, C]
        m["xT"] = np.ascontiguousarray(xb.T, dtype=f32)
        m["xF"] = np.ascontiguousarray(
            xb.T.reshape(2, 128, N).transpose(1, 0, 2)).astype(f8)
        m["ct"] = np.ascontiguousarray(ct128[:, rot]).astype(bf)
        m["st"] = np.ascontiguousarray(st128[:, rot]).astype(bf)
        in_maps.append(m)
    return in_maps


def kernel(x, Wqkv, Wproj, g1, b1, g2, b2, W1, bf1, W2, bf2, H, W, **kw):
    from concourse.bass_utils import run_bass_kernel_spmd

    x = np.asarray(x, dtype=np.float32)
    args = [np.asarray(a, dtype=np.float32)
            for a in (Wqkv, Wproj, g1, b1, g2, b2, W1, bf1, W2, bf2)]
    H, W = int(H), int(W)

    with_b2 = bool(np.any(args[5]))
    nc = _get_program(with_b2, _CACHE.get("debug", False))
    in_maps = _host_prep(x, *args, H, W)
    res = run_bass_kernel_spmd(nc, in_maps, core_ids=list(range(NCORES)),
                               **_CACHE.get("run_kwargs", {}))
    _CACHE["last_result"] = res

    out = np.zeros((B, N, C), dtype=np.float32)
    for core in range(NCORES):
        b, qh = core // 2, core % 2
        n0 = qh * NQ
        out[b, n0 : n0 + NQ, :] = res.results[core]["out"].astype(np.float32)
    return out


# revision 12
# speedup vs baseline: 1.1312x; 1.0007x over previous
"""Self-contained Trainium2 Bass kernel for the AttnBlock problem — fp8 edition.

Sharding: 8 cores; core c handles batch b = c//2, query rows
[qh*1152, (qh+1)*1152) with qh = c%2.  Each core computes full K/V for its
batch (duplicated across the 2 cores of a batch) so there are NO collectives.
Per-core token order is rotated so each core's own query tokens are always
columns 0..1151.

All heavy matmuls run in fp8e4m3 with DoubleRow perf mode (2 contraction
subtiles per pass, 0.5 cycles/column).  Weights are host-scaled by 32 into
the fp8 sweet spot; activations carry powers-of-two scales that are folded
into downstream scalars:
  qrot/krot = bf16(32*rope(q|k)) (scores stay bf16 — contraction is 64, so
  fp8 wins nothing there), scores PSUM = 8192*s (s = q.k/sqrt(64))
  exp: 8*exp(s) as fp8 — on ACT (Exp table, scale 1/8192, bias ln8) or on
  DVE via the direct fp8-byte trick u8 = round(8*log2(e)*s + 80).
  v_allF = fp8(32*v) with a 32-valued ones row at index 64 (lhsT padded to
  M=128; rows 65..127 are never read downstream).
  attnF = fp8(32*attn_out); proj PSUM = 1024*proj.
  zgF = fp8(LN1 out); FFN1 PSUM = 32*h; hts = fp8(gelu(h));
  FFN2 PSUM = 32*(ffn2 + zg) via an extra 32*identity bf16 matmul.

LayerNorm rstd = exp(-0.5*ln(var+eps)) — Ln and Exp share one ACT table
with the attention exps, so only Gelu causes table reloads.
"""

import numpy as np

B, N, C = 4, 2304, 256
NH, DH = 4, 64
HALF = DH // 2         # 32 (rope half)
NQ = N // 2            # 1152
F = 4 * C              # 1024
NCORES = 8
MT = N // 128          # 18 key tiles
MG2 = MT // 2          # 9 key-tile pairs
EPS = 1e-5
LOG2E8 = 11.541560327111707   # 8*log2(e)
EXPBIAS = 80.0                # (7+3)*8: fp8 byte bias for 8*exp(s)
JW = [(0, 512), (512, 512), (1024, 128)]
NJ = 3
EXP_DVE_G = (4,)       # key-tile-pair indices whose exp runs on DVE

_CACHE = {}


def _build_program(with_b2, debug=False):
    import concourse.tile as tile
    from concourse import bacc, mybir
    from concourse.masks import make_identity

    f32 = mybir.dt.float32
    f32r = mybir.dt.float32r
    bf16 = mybir.dt.bfloat16
    fp8 = mybir.dt.float8e4
    u8 = mybir.dt.uint8
    Alu = mybir.AluOpType
    Act = mybir.ActivationFunctionType
    DR = mybir.MatmulPerfMode.DoubleRow

    nc = bacc.Bacc(None, target_bir_lowering=False, debug=False)

    def dram(name, shape, dt=f32, out=False):
        return nc.dram_tensor(
            name, list(shape), dt, kind="ExternalOutput" if out else "ExternalInput"
        )

    d_xT = dram("xT", [C, N])
    d_xF = dram("xF", [128, 2, N], fp8)
    d_ct = dram("ct", [128, N], bf16)
    d_st = dram("st", [128, N], bf16)
    d_wq = dram("wq", [128, 2, 256], fp8)
    d_wqs = dram("wqs", [128, 2, 256], fp8)
    d_wk = dram("wk", [128, 2, 256], fp8)
    d_wks = dram("wks", [128, 2, 256], fp8)
    d_wv = dram("wv", [128, 2, 256], fp8)
    d_wp = dram("wp", [128, 2, 256], fp8)
    d_w1 = dram("w1", [128, 2, F], fp8)
    d_w2 = dram("w2", [128, 4, 2, 256], fp8)
    d_g1 = dram("g1c", [C, 1])
    d_g2 = dram("g2c", [C, 1])
    d_B2 = dram("B2c", [C, 1])
    d_b2 = dram("b2c", [C, 1])
    d_bf1 = dram("bf1c", [F, 1])
    d_out = nc.dram_tensor("out", [NQ, C], bf16, kind="ExternalOutput")
    if debug:
        d_dbg = {
            "dbg_qrot0": nc.dram_tensor("dbg_qrot0", [128, NQ], bf16, kind="ExternalOutput"),
            "dbg_qrot1": nc.dram_tensor("dbg_qrot1", [128, NQ], bf16, kind="ExternalOutput"),
            "dbg_krot0": nc.dram_tensor("dbg_krot0", [128, N], bf16, kind="ExternalOutput"),
            "dbg_krot1": nc.dram_tensor("dbg_krot1", [128, N], bf16, kind="ExternalOutput"),
            "dbg_v": dram("dbg_v", [128, MG2, 2, NH, 128], fp8, out=True),
            "dbg_attnF": dram("dbg_attnF", [128, 2, NQ], fp8, out=True),
            "dbg_zgF": dram("dbg_zgF", [128, 2, NQ], fp8, out=True),
        }

    mm = nc.tensor.matmul

    with tile.TileContext(nc) as tc:
        with tc.tile_pool(name="persist", bufs=1) as P:
            xT = [P.tile([128, N], f32, name=f"xT{i}") for i in range(2)]
            xF = P.tile([128, 2, N], fp8, name="xF")
            ct = P.tile([128, N], bf16, name="ct")
            st = P.tile([128, N], bf16, name="st")
            wq = P.tile([128, 2, 256], fp8, name="wq")
            wqs = P.tile([128, 2, 256], fp8, name="wqs")
            wk = P.tile([128, 2, 256], fp8, name="wk")
            wks = P.tile([128, 2, 256], fp8, name="wks")
            wv = P.tile([128, 2, 256], fp8, name="wv")
            wp = P.tile([128, 2, 256], fp8, name="wp")
            w1 = P.tile([128, 2, F], fp8, name="w1")
            w2 = P.tile([128, 4, 2, 256], fp8, name="w2")
            g1c = [P.tile([128, 1], f32, name=f"g1c{i}") for i in range(2)]
            g2c = [P.tile([128, 1], f32, name=f"g2c{i}") for i in range(2)]
            B2c = [P.tile([128, 1], f32, name=f"B2c{i}") for i in range(2)]
            b2c = [P.tile([128, 1], f32, name=f"b2c{i}") for i in range(2)]
            bf1c = P.tile([128, 8], f32, name="bf1c")
            ln8t = P.tile([128, 1], f32, name="ln8t")
            zerot = P.tile([128, 1], f32, name="zerot")
            epst = P.tile([128, 1], f32, name="epst")
            identb = P.tile([128, 128], bf16, name="identb")
            ones = P.tile([128, 1], f32r, name="ones")
            qrot = [P.tile([128, NQ], bf16, name=f"qrot{i}") for i in range(2)]
            krot = [P.tile([128, N], bf16, name=f"krot{i}") for i in range(2)]
            v_allF = P.tile([128, MG2, 2, NH, 128], fp8, name="v_allF")
            attnF = P.tile([128, 2, NQ], fp8, name="attnF")
            zgF = P.tile([128, 2, NQ], fp8, name="zgF")

            # critical-path loads first
            nc.sync.dma_start(xF, d_xF[:, :, :])
            nc.sync.dma_start(ct, d_ct[:, :])
            nc.sync.dma_start(st, d_st[:, :])
            for t, d in [(wq, d_wq), (wqs, d_wqs), (wk, d_wk), (wks, d_wks),
                         (wv, d_wv)]:
                nc.sync.dma_start(t, d[:, :, :])
            nc.sync.dma_start(wp, d_wp[:, :, :])
            nc.sync.dma_start(w1, d_w1[:, :, :])
            nc.sync.dma_start(w2, d_w2[:, :, :, :])
            for i in range(2):
                nc.sync.dma_start(g1c[i], d_g1[i * 128 : (i + 1) * 128, :])
                nc.sync.dma_start(g2c[i], d_g2[i * 128 : (i + 1) * 128, :])
                nc.sync.dma_start(B2c[i], d_B2[i * 128 : (i + 1) * 128, :])
                nc.sync.dma_start(b2c[i], d_b2[i * 128 : (i + 1) * 128, :])
                nc.sync.dma_start(xT[i], d_xT[i * 128 : (i + 1) * 128, :])
            for i in range(8):
                nc.sync.dma_start(bf1c[:, i : i + 1], d_bf1[i * 128 : (i + 1) * 128, :])

            onesf = P.tile([128, 1], f32, name="onesf")
            nc.vector.memset(onesf, 1.0)
            nc.vector.tensor_copy(ones, onesf)
            nc.vector.memset(ln8t, float(np.log(8.0)))
            nc.vector.memset(zerot, 0.0)
            nc.vector.memset(epst, EPS)
            identf = P.tile([128, 128], f32, name="identf")
            make_identity(nc, identf)
            nc.vector.tensor_copy(identb, identf)
            # ones row of v lhsT (value 32); zero the M-padding rows 65..127
            nc.vector.memset(v_allF[:, :, :, :, DH : DH + 1], 32.0)
            nc.vector.memset(v_allF[:, :, :, :, DH + 1 : 128], 0.0)

            with (
                tc.tile_pool(name="psCD", bufs=1, space="PSUM") as pP,
                tc.tile_pool(name="sbCD", bufs=1) as sD,
            ):
                # ---------- phase A: q/k rope projections + v ----------
                def rope_proj(dst, w_pre, w_shf, chunks, tag):
                    for cc in range(2):
                        for (ofs, W_) in chunks:
                            sl = slice(ofs, ofs + W_)
                            pre = pP.tile([128, 512], f32, tag="sc", bufs=2,
                                          name=f"pre{tag}{cc}{ofs}")
                            mm(pre[:, 0:W_], w_pre[:, :, cc * 128 : (cc + 1) * 128],
                               xF[:, :, sl], start=True, stop=True, perf_mode=DR)
                            shf = pP.tile([128, 512], f32, tag="sc", bufs=2,
                                          name=f"shf{tag}{cc}{ofs}")
                            mm(shf[:, 0:W_], w_shf[:, :, cc * 128 : (cc + 1) * 128],
                               xF[:, :, sl], start=True, stop=True, perf_mode=DR)
                            t1 = sD.tile([128, 512], f32, tag="t1", bufs=2, name="t1")
                            t2 = sD.tile([128, 512], f32, tag="t2", bufs=2, name="t2")
                            nc.vector.tensor_mul(t1[:, 0:W_], pre[:, 0:W_], ct[:, sl])
                            nc.vector.tensor_mul(t2[:, 0:W_], shf[:, 0:W_], st[:, sl])
                            nc.gpsimd.tensor_add(dst[cc][:, sl],
                                                 t1[:, 0:W_], t2[:, 0:W_])

                QCH = [(0, 512), (512, 512), (1024, 128)]
                KCH = [(o, min(512, N - o)) for o in range(0, N, 512)]
                rope_proj(qrot, wq, wqs, QCH[:1], "q0")
                rope_proj(krot, wk, wks, KCH, "k")
                for m in range(MT):
                    g, sv = divmod(m, 2)
                    psv = pP.tile([128, 256], f32, tag="at", bufs=2, name=f"psv{m}")
                    mm(psv, xF[:, :, m * 128 : (m + 1) * 128], wv,
                       start=True, stop=True, perf_mode=DR)
                    nc.vector.tensor_copy(
                        v_allF[:, g, sv, :, 0:DH],
                        psv.rearrange("p (h d) -> p h d", h=NH),
                    )
                rope_proj(qrot, wq, wqs, QCH[1:], "q1")

                # ---------- per-j: attention, proj+LN1, FFN, LN2, out ----------
                def attn_head(j, h):
                    ofs, W_ = JW[j]
                    jsl = slice(ofs, ofs + W_)
                    hc, hp = h // 2, slice((h % 2) * 64, (h % 2) * 64 + 64)
                    at = pP.tile([128, 512], f32, tag="at", bufs=2, name=f"at{j}{h}")
                    for g in range(MG2):
                        scp = pP.tile([128, 2, 512], f32, tag="scp", bufs=2,
                                      name=f"sc{j}{h}{g}")
                        for si in range(2):
                            m = 2 * g + si
                            mm(scp[:, si, 0:W_],
                               krot[hc][hp, m * 128 : (m + 1) * 128],
                               qrot[hc][hp, jsl], start=True, stop=True)
                        exf = sD.tile([128, 2, 512], fp8, tag="ex", bufs=6,
                                      name=f"ex{j}{h}{g}")
                        if g in EXP_DVE_G:
                            nc.vector.tensor_scalar(
                                exf.bitcast(u8)[:, :, 0:W_], scp[:, :, 0:W_],
                                LOG2E8 / 8192.0, EXPBIAS, Alu.mult, Alu.add)
                        else:
                            nc.scalar.activation(exf[:, :, 0:W_], scp[:, :, 0:W_],
                                                 Act.Exp, scale=1.0 / 8192.0,
                                                 bias=ln8t[:, :])
                        mm(at[:, 0:W_], v_allF[:, g, :, h, :], exf[:, :, 0:W_],
                           start=(g == 0), stop=(g == MG2 - 1), perf_mode=DR)
                    denr = sD.tile([1, 512], f32, tag="row", bufs=6,
                                   name=f"denr{j}{h}")
                    nc.vector.tensor_copy(denr[:, 0:W_], at[DH : DH + 1, 0:W_])
                    den = sD.tile([1, 512], f32, tag="row", bufs=6, name=f"den{j}{h}")
                    nc.vector.reciprocal_approx_fast(den[:, 0:W_], denr[:, 0:W_])
                    rb = sD.tile([64, 512], f32, tag="rb", bufs=2, name=f"rb{j}{h}")
                    nc.gpsimd.partition_broadcast(rb[:, 0:W_], den[0:1, 0:W_])
                    nc.vector.scalar_tensor_tensor(
                        attnF[(h % 2) * 64 : (h % 2) * 64 + 64, h // 2, jsl],
                        at[0:DH, 0:W_], 32.0, rb[:, 0:W_], Alu.mult, Alu.mult)

                def ln_rows(pair, W_, tagp):
                    """-> (sum_b, rs): raw column-sum broadcast and 1/std bcast."""
                    pssum = pP.tile([1, 512], f32, tag="at", bufs=2,
                                    name=f"pssum{tagp}")
                    for co in range(2):
                        mm(pssum[:, 0:W_], ones, pair[co],
                           start=(co == 0), stop=(co == 1))
                    pssq = pP.tile([1, 512], f32, tag="at", bufs=2,
                                   name=f"pssq{tagp}")
                    for co in range(2):
                        sq = sD.tile([128, 512], f32r, tag="sq", bufs=2,
                                     name=f"sq{tagp}{co}")
                        nc.vector.tensor_mul(sq[:, 0:W_], pair[co], pair[co])
                        mm(pssq[:, 0:W_], ones, sq[:, 0:W_],
                           start=(co == 0), stop=(co == 1))
                    sumr = sD.tile([1, 512], f32, tag="row", bufs=6,
                                   name=f"sumr{tagp}")
                    nc.vector.tensor_copy(sumr[:, 0:W_], pssum[:, 0:W_])
                    u = sD.tile([1, 512], f32, tag="row", bufs=6, name=f"u{tagp}")
                    nc.vector.scalar_tensor_tensor(u[:, 0:W_], sumr[:, 0:W_], 1.0 / C,
                                                   sumr[:, 0:W_], Alu.mult, Alu.mult)
                    w_ = sD.tile([1, 512], f32, tag="row", bufs=6, name=f"w{tagp}")
                    nc.vector.tensor_sub(w_[:, 0:W_], pssq[:, 0:W_], u[:, 0:W_])
                    # rstd = 1/sqrt(var+eps) on DVE: magic-seed + 1 Newton step
                    i32 = mybir.dt.int32
                    vv = sD.tile([1, 512], f32, tag="row", bufs=6, name=f"vv{tagp}")
                    nc.vector.tensor_scalar(vv[:, 0:W_], w_[:, 0:W_], 1.0 / C, EPS,
                                            Alu.mult, Alu.add)
                    hi = sD.tile([1, 512], f32, tag="row", bufs=6,
                                 name=f"hi{tagp}")
                    nc.vector.tensor_scalar(hi.bitcast(i32)[:, 0:W_],
                                            vv.bitcast(i32)[:, 0:W_],
                                            1, None, Alu.arith_shift_right)
                    y0i = sD.tile([1, 512], f32, tag="row", bufs=6,
                                  name=f"y0i{tagp}")
                    nc.vector.tensor_scalar(y0i.bitcast(i32)[:, 0:W_],
                                            hi.bitcast(i32)[:, 0:W_],
                                            -1, 0x5F3759DF, Alu.mult, Alu.add)
                    t_ = sD.tile([1, 512], f32, tag="row", bufs=6, name=f"t{tagp}")
                    nc.vector.tensor_mul(t_[:, 0:W_], y0i[:, 0:W_], y0i[:, 0:W_])
                    nc.vector.tensor_mul(t_[:, 0:W_], t_[:, 0:W_], vv[:, 0:W_])
                    nc.vector.tensor_scalar(t_[:, 0:W_], t_[:, 0:W_], -0.5, 1.5,
                                            Alu.mult, Alu.add)
                    rstd = sD.tile([1, 512], f32, tag="row", bufs=6,
                                   name=f"rstd{tagp}")
                    nc.vector.tensor_mul(rstd[:, 0:W_], y0i[:, 0:W_], t_[:, 0:W_])
                    sum_b = sD.tile([128, 512], f32, tag="bc", bufs=4,
                                    name=f"sumb{tagp}")
                    nc.gpsimd.partition_broadcast(sum_b[:, 0:W_], sumr[0:1, 0:W_])
                    rs = sD.tile([128, 512], f32, tag="bc", bufs=4, name=f"rs{tagp}")
                    nc.gpsimd.partition_broadcast(rs[:, 0:W_], rstd[0:1, 0:W_])
                    return sum_b, rs

                def d_proj_ln1(j):
                    ofs, W_ = JW[j]
                    jsl = slice(ofs, ofs + W_)
                    res = []
                    for co in range(2):
                        psp = pP.tile([128, 512], f32, tag="sc", bufs=2,
                                      name=f"psp{j}{co}")
                        mm(psp[:, 0:W_], wp[:, :, co * 128 : (co + 1) * 128],
                           attnF[:, :, jsl], start=True, stop=True, perf_mode=DR)
                        rt = sD.tile([128, 512], f32r, tag="res", bufs=4,
                                     name=f"res{j}{co}")
                        nc.vector.scalar_tensor_tensor(rt[:, 0:W_], psp[:, 0:W_],
                                                       1.0 / 1024.0, xT[co][:, jsl],
                                                       Alu.mult, Alu.add)
                        res.append(rt[:, 0:W_])
                    sum_b, rs = ln_rows(res, W_, f"a{j}")
                    zgs = []
                    for co in range(2):
                        A = sD.tile([128, 512], f32, tag="za", bufs=2,
                                    name=f"A{j}{co}")
                        nc.vector.scalar_tensor_tensor(A[:, 0:W_], sum_b[:, 0:W_],
                                                       -1.0 / C, res[co],
                                                       Alu.mult, Alu.add)
                        z = sD.tile([128, 512], bf16, tag="zg", bufs=4,
                                    name=f"zg{j}{co}")
                        nc.vector.scalar_tensor_tensor(z[:, 0:W_], A[:, 0:W_],
                                                       g1c[co], rs[:, 0:W_],
                                                       Alu.mult, Alu.mult)
                        nc.gpsimd.tensor_copy(zgF[:, co, jsl], z[:, 0:W_])
                        zgs.append(z[:, 0:W_])
                    return zgs

                def d_ffn1(j):
                    ofs, W_ = JW[j]
                    jsl = slice(ofs, ofs + W_)
                    hts = sD.tile([128, 8, 512], fp8, tag="hts", bufs=2,
                                  name=f"hts{j}")
                    for f in range(8):
                        psh = pP.tile([128, 512], f32, tag="sc", bufs=2,
                                      name=f"psh{j}{f}")
                        mm(psh[:, 0:W_], w1[:, :, f * 128 : (f + 1) * 128],
                           zgF[:, :, jsl], start=True, stop=True, perf_mode=DR)
                        nc.scalar.activation(hts[:, f, 0:W_], psh[:, 0:W_], Act.Gelu,
                                             bias=bf1c[:, f : f + 1], scale=1.0 / 32.0)
                    return hts

                def d_ffn2_ln2_out(j, zgs, hts):
                    ofs, W_ = JW[j]
                    x2l = []
                    for co in range(2):
                        psf = pP.tile([128, 512], f32, tag="sc", bufs=2,
                                      name=f"psf{j}{co}")
                        for g2 in range(4):
                            mm(psf[:, 0:W_], w2[:, g2, :, co * 128 : (co + 1) * 128],
                               hts[:, 2 * g2 : 2 * g2 + 2, 0:W_],
                               start=(g2 == 0), stop=(g2 == 3), perf_mode=DR)
                        x2t = sD.tile([128, 512], f32, tag="za", bufs=2,
                                      name=f"x2t{j}{co}")
                        nc.vector.tensor_scalar(x2t[:, 0:W_], psf[:, 0:W_],
                                                1.0 / 32.0, B2c[co],
                                                Alu.mult, Alu.add)
                        x2 = sD.tile([128, 512], f32r, tag="x2", bufs=4,
                                     name=f"x2_{j}{co}")
                        nc.vector.tensor_add(x2[:, 0:W_], x2t[:, 0:W_], zgs[co])
                        x2l.append(x2[:, 0:W_])
                    sum_b2, rs2 = ln_rows(x2l, W_, f"b{j}")
                    for co in range(2):
                        A2 = sD.tile([128, 512], f32, tag="za", bufs=2,
                                     name=f"A2_{j}{co}")
                        nc.vector.scalar_tensor_tensor(A2[:, 0:W_], sum_b2[:, 0:W_],
                                                       -1.0 / C, x2l[co],
                                                       Alu.mult, Alu.add)
                        fz = sD.tile([128, 512], bf16, tag="fz", bufs=2,
                                     name=f"fz{j}{co}")
                        nc.vector.scalar_tensor_tensor(fz[:, 0:W_], A2[:, 0:W_],
                                                       g2c[co], rs2[:, 0:W_],
                                                       Alu.mult, Alu.mult)
                        if with_b2:
                            fo = sD.tile([128, 512], bf16, tag="fz", bufs=2,
                                         name=f"fo{j}{co}")
                            nc.gpsimd.tensor_scalar_add(fo[:, 0:W_], fz[:, 0:W_],
                                                        b2c[co])
                            fz = fo
                        x2l[co] = fz
                    for tt in range(W_ // 128):
                        t0 = ofs + tt * 128
                        tok = sD.tile([128, C], bf16, tag="tok", bufs=3,
                                      name=f"tok{j}{tt}")
                        for co in range(2):
                            pst = pP.tile([128, 128], bf16, tag="at", bufs=2,
                                          name=f"pst{j}{tt}{co}")
                            nc.tensor.transpose(
                                pst, x2l[co][:, tt * 128 : (tt + 1) * 128], identb
                            )
                            csl = slice(co * 128, (co + 1) * 128)
                            if co == 0:
                                nc.vector.tensor_copy(tok[:, csl], pst)
                            else:
                                nc.scalar.copy(tok[:, csl], pst)
                        nc.sync.dma_start(d_out[t0 : t0 + 128, :], tok)

                for j in range(NJ):
                    for h in range(NH):
                        attn_head(j, h)
                    zgs = d_proj_ln1(j)
                    hts = d_ffn1(j)
                    d_ffn2_ln2_out(j, zgs, hts)

                if debug:
                    for i in range(2):
                        nc.sync.dma_start(d_dbg["dbg_qrot%d" % i][:, :], qrot[i])
                        nc.sync.dma_start(d_dbg["dbg_krot%d" % i][:, :], krot[i])
                    nc.sync.dma_start(d_dbg["dbg_v"][:, :, :, :, :], v_allF)
                    nc.sync.dma_start(d_dbg["dbg_attnF"][:, :, :], attnF)
                    nc.sync.dma_start(d_dbg["dbg_zgF"][:, :, :], zgF)

    nc.compile()
    return nc


def _get_program(with_b2, debug=False):
    key = f"nc{int(with_b2)}_{int(debug)}"
    if key not in _CACHE:
        _CACHE[key] = _build_program(with_b2, debug)
    return _CACHE[key]


def _host_prep(x, Wqkv, Wproj, g1, b1, g2, b2, W1, bf1, W2, bf2, H, W):
    import ml_dtypes

    f8 = ml_dtypes.float8_e4m3
    bf = ml_dtypes.bfloat16
    f32 = np.float32

    Wq, Wk, Wv = Wqkv[0:C], Wqkv[C : 2 * C], Wqkv[2 * C : 3 * C]

    # rotate-half partner: pair-swap within each 32-dim rope half (d ^ 1)
    permC = np.arange(C) ^ 1

    def proj_tiles(Wm):
        """-> (pre, shf) [128, 2, 256] fp8, x32, out cols in original order."""
        pre = (Wm * 32.0).T.reshape(2, 128, 256).transpose(1, 0, 2)
        shf = (Wm[permC] * 32.0).T.reshape(2, 128, 256).transpose(1, 0, 2)
        return (np.ascontiguousarray(pre).astype(f8),
                np.ascontiguousarray(shf).astype(f8))

    wq8, wqs8 = proj_tiles(Wq)
    wk8, wks8 = proj_tiles(Wk)
    wv8 = np.ascontiguousarray((Wv * 32.0).T.reshape(2, 128, 256)
                               .transpose(1, 0, 2)).astype(f8)
    wp8 = np.ascontiguousarray((Wproj * 32.0).T.reshape(2, 128, 256)
                               .transpose(1, 0, 2)).astype(f8)
    w18 = np.ascontiguousarray((W1 * 32.0).T.reshape(2, 128, F)
                               .transpose(1, 0, 2)).astype(f8)
    # w2 [128, 4, 2, 256]: in-feature (g2, s, p) = (2*g2+s)*128+p
    w28 = np.ascontiguousarray((W2 * 32.0).T.reshape(4, 2, 128, C)
                               .transpose(2, 0, 1, 3)).astype(f8)

    shared = {
        "wq": wq8, "wqs": wqs8, "wk": wk8, "wks": wks8, "wv": wv8, "wp": wp8,
        "w1": w18, "w2": w28,
        "g1c": np.ascontiguousarray(g1.reshape(C, 1), dtype=f32),
        "g2c": np.ascontiguousarray(g2.reshape(C, 1), dtype=f32),
        "B2c": np.ascontiguousarray((b1 + bf2).reshape(C, 1), dtype=f32),
        "b2c": np.ascontiguousarray(b2.reshape(C, 1), dtype=f32),
        "bf1c": np.ascontiguousarray((bf1 + W1 @ b1).reshape(F, 1), dtype=f32),
    }

    # rope tables, baseline row layout: row d (of 64-dim head block) x 2
    invf = 1.0 / (10000.0 ** (np.arange(HALF, dtype=np.float64) / HALF))
    yy, xx = np.meshgrid(np.arange(H), np.arange(W), indexing="ij")
    pos_y = yy.reshape(-1).astype(np.float64)
    pos_x = xx.reshape(-1).astype(np.float64)
    ang = np.concatenate(
        [invf[:, None] * pos_y[None, :], invf[:, None] * pos_x[None, :]], axis=0
    )  # [64, N]
    sgn = np.where(np.arange(DH) % 2 == 0, -1.0, 1.0)[:, None]
    ct64 = np.cos(ang)
    st64 = np.sin(ang) * sgn
    ct128 = np.concatenate([ct64, ct64], axis=0)  # [128, N]
    st128 = np.concatenate([st64, st64], axis=0)

    in_maps = []
    for core in range(NCORES):
        b, qh = core // 2, core % 2
        n0 = qh * NQ
        rot = np.concatenate([np.arange(n0, N), np.arange(0, n0)])
        m = dict(shared)
        xb = x[b][rot]                                          # [N, C]
        m["xT"] = np.ascontiguousarray(xb.T, dtype=f32)
        m["xF"] = np.ascontiguousarray(
            xb.T.reshape(2, 128, N).transpose(1, 0, 2)).astype(f8)
        m["ct"] = np.ascontiguousarray(ct128[:, rot]).astype(bf)
        m["st"] = np.ascontiguousarray(st128[:, rot]).astype(bf)
        in_maps.append(m)
    return in_maps


def kernel(x, Wqkv, Wproj, g1, b1, g2, b2, W1, bf1, W2, bf2, H, W, **kw):
    from concourse.bass_utils import run_bass_kernel_spmd

    x = np.asarray(x, dtype=np.float32)
    args = [np.asarray(a, dtype=np.float32)
            for a in (Wqkv, Wproj, g1, b1, g2, b2, W1, bf1, W2, bf2)]
    H, W = int(H), int(W)

    with_b2 = bool(np.any(args[5]))
    nc = _get_program(with_b2, _CACHE.get("debug", False))
    in_maps = _host_prep(x, *args, H, W)
    res = run_bass_kernel_spmd(nc, in_maps, core_ids=list(range(NCORES)),
                               **_CACHE.get("run_kwargs", {}))
    _CACHE["last_result"] = res

    out = np.zeros((B, N, C), dtype=np.float32)
    for core in range(NCORES):
        b, qh = core // 2, core % 2
        n0 = qh * NQ
        out[b, n0 : n0 + NQ, :] = res.results[core]["out"].astype(np.float32)
    return out


# revision 13
# speedup vs baseline: 1.2487x; 1.1038x over previous
"""Self-contained Trainium2 Bass kernel for the AttnBlock problem — fp8 edition.

Sharding: 8 cores; core c handles batch b = c//2, query rows
[qh*1152, (qh+1)*1152) with qh = c%2.  Each core computes full K/V for its
batch (duplicated across the 2 cores of a batch) so there are NO collectives.
Per-core token order is rotated so each core's own query tokens are always
columns 0..1151.

All heavy matmuls run in fp8e4m3 with DoubleRow perf mode (2 contraction
subtiles per pass, 0.5 cycles/column).  Weights are host-scaled by 32 into
the fp8 sweet spot; activations carry powers-of-two scales that are folded
into downstream scalars:
  qrot/krot = bf16(32*rope(q|k)) (scores stay bf16 — contraction is 64, so
  fp8 wins nothing there), scores PSUM = 8192*s (s = q.k/sqrt(64))
  exp: 8*exp(s) as fp8 — on ACT (Exp table, scale 1/8192, bias ln8) or on
  DVE via the direct fp8-byte trick u8 = round(8*log2(e)*s + 80).
  v_allF = fp8(32*v) with a 32-valued ones row at index 64 (lhsT padded to
  M=128; rows 65..127 are never read downstream).
  attnF = fp8(32*attn_out); proj PSUM = 1024*proj.
  zgF = fp8(LN1 out); FFN1 PSUM = 32*h; hts = fp8(gelu(h));
  FFN2 PSUM = 32*(ffn2 + zg) via an extra 32*identity bf16 matmul.

LayerNorm rstd = exp(-0.5*ln(var+eps)) — Ln and Exp share one ACT table
with the attention exps, so only Gelu causes table reloads.
"""

import numpy as np

B, N, C = 4, 2304, 256
NH, DH = 4, 64
HALF = DH // 2         # 32 (rope half)
NQ = N // 2            # 1152
F = 4 * C              # 1024
NCORES = 8
MT = N // 128          # 18 key tiles
MG2 = MT // 2          # 9 key-tile pairs
EPS = 1e-5
LOG2E8 = 11.541560327111707   # 8*log2(e)
EXPBIAS = 80.0                # (7+3)*8: fp8 byte bias for 8*exp(s)
JW = [(0, 512), (512, 512), (1024, 128)]
NJ = 3
EXP_DVE_G = (4,)       # key-tile-pair indices whose exp runs on DVE

_CACHE = {}


def _build_program(with_b2, debug=False):
    import concourse.tile as tile
    from concourse import bacc, mybir
    from concourse.masks import make_identity

    f32 = mybir.dt.float32
    f32r = mybir.dt.float32r
    bf16 = mybir.dt.bfloat16
    fp8 = mybir.dt.float8e4
    u8 = mybir.dt.uint8
    Alu = mybir.AluOpType
    Act = mybir.ActivationFunctionType
    DR = mybir.MatmulPerfMode.DoubleRow

    nc = bacc.Bacc(None, target_bir_lowering=False, debug=False)

    def dram(name, shape, dt=f32, out=False):
        return nc.dram_tensor(
            name, list(shape), dt, kind="ExternalOutput" if out else "ExternalInput"
        )

    d_xT = dram("xT", [C, N])
    d_xF = dram("xF", [128, 2, N], fp8)
    d_ct = dram("ct", [128, N], bf16)
    d_st = dram("st", [128, N], bf16)
    d_wq = dram("wq", [128, 2, 256], fp8)
    d_wqs = dram("wqs", [128, 2, 256], fp8)
    d_wk = dram("wk", [128, 2, 256], fp8)
    d_wks = dram("wks", [128, 2, 256], fp8)
    d_wv = dram("wv", [128, 2, 256], fp8)
    d_wp = dram("wp", [128, 2, 256], fp8)
    d_w1 = dram("w1", [128, 2, F], fp8)
    d_w2 = dram("w2", [128, 4, 2, 256], fp8)
    d_g1 = dram("g1c", [C, 1])
    d_g2 = dram("g2c", [C, 1])
    d_B2 = dram("B2c", [C, 1])
    d_b2 = dram("b2c", [C, 1])
    d_bf1 = dram("bf1c", [F, 1])
    d_out = nc.dram_tensor("out", [NQ, C], bf16, kind="ExternalOutput")
    if debug:
        d_dbg = {
            "dbg_qrot0": nc.dram_tensor("dbg_qrot0", [128, NQ], bf16, kind="ExternalOutput"),
            "dbg_qrot1": nc.dram_tensor("dbg_qrot1", [128, NQ], bf16, kind="ExternalOutput"),
            "dbg_krot0": nc.dram_tensor("dbg_krot0", [128, N], bf16, kind="ExternalOutput"),
            "dbg_krot1": nc.dram_tensor("dbg_krot1", [128, N], bf16, kind="ExternalOutput"),
            "dbg_v": dram("dbg_v", [128, MG2, 2, NH, 128], fp8, out=True),
            "dbg_attnF": dram("dbg_attnF", [128, 2, NQ], fp8, out=True),
            "dbg_zgF": dram("dbg_zgF", [128, 2, NQ], fp8, out=True),
        }

    mm = nc.tensor.matmul

    with tile.TileContext(nc) as tc:
        with tc.tile_pool(name="persist", bufs=1) as P:
            xT = [P.tile([128, N], f32, name=f"xT{i}") for i in range(2)]
            xF = P.tile([128, 2, N], fp8, name="xF")
            ct = P.tile([128, N], bf16, name="ct")
            st = P.tile([128, N], bf16, name="st")
            wq = P.tile([128, 2, 256], fp8, name="wq")
            wqs = P.tile([128, 2, 256], fp8, name="wqs")
            wk = P.tile([128, 2, 256], fp8, name="wk")
            wks = P.tile([128, 2, 256], fp8, name="wks")
            wv = P.tile([128, 2, 256], fp8, name="wv")
            wp = P.tile([128, 2, 256], fp8, name="wp")
            w1 = P.tile([128, 2, F], fp8, name="w1")
            w2 = P.tile([128, 4, 2, 256], fp8, name="w2")
            g1c = [P.tile([128, 1], f32, name=f"g1c{i}") for i in range(2)]
            g2c = [P.tile([128, 1], f32, name=f"g2c{i}") for i in range(2)]
            B2c = [P.tile([128, 1], f32, name=f"B2c{i}") for i in range(2)]
            b2c = [P.tile([128, 1], f32, name=f"b2c{i}") for i in range(2)]
            bf1c = P.tile([128, 8], f32, name="bf1c")
            ln8t = P.tile([128, 1], f32, name="ln8t")
            zerot = P.tile([128, 1], f32, name="zerot")
            epst = P.tile([128, 1], f32, name="epst")
            identb = P.tile([128, 128], bf16, name="identb")
            ones = P.tile([128, 1], f32r, name="ones")
            qrot = [P.tile([128, NQ], bf16, name=f"qrot{i}") for i in range(2)]
            krot = [P.tile([128, N], bf16, name=f"krot{i}") for i in range(2)]
            v_allF = P.tile([128, MG2, 2, NH, 128], fp8, name="v_allF")
            attnF = P.tile([128, 2, NQ], fp8, name="attnF")
            zgF = P.tile([128, 2, NQ], fp8, name="zgF")

            # critical-path loads first
            nc.sync.dma_start(xF, d_xF[:, :, :])
            nc.sync.dma_start(ct, d_ct[:, :])
            nc.sync.dma_start(st, d_st[:, :])
            for t, d in [(wq, d_wq), (wqs, d_wqs), (wk, d_wk), (wks, d_wks),
                         (wv, d_wv)]:
                nc.sync.dma_start(t, d[:, :, :])
            nc.sync.dma_start(wp, d_wp[:, :, :])
            nc.sync.dma_start(w1, d_w1[:, :, :])
            nc.sync.dma_start(w2, d_w2[:, :, :, :])
            for i in range(2):
                nc.sync.dma_start(g1c[i], d_g1[i * 128 : (i + 1) * 128, :])
                nc.sync.dma_start(g2c[i], d_g2[i * 128 : (i + 1) * 128, :])
                nc.sync.dma_start(B2c[i], d_B2[i * 128 : (i + 1) * 128, :])
                nc.sync.dma_start(b2c[i], d_b2[i * 128 : (i + 1) * 128, :])
                nc.sync.dma_start(xT[i], d_xT[i * 128 : (i + 1) * 128, :])
            for i in range(8):
                nc.sync.dma_start(bf1c[:, i : i + 1], d_bf1[i * 128 : (i + 1) * 128, :])

            onesf = P.tile([128, 1], f32, name="onesf")
            nc.vector.memset(onesf, 1.0)
            nc.vector.tensor_copy(ones, onesf)
            nc.vector.memset(ln8t, float(np.log(8.0)))
            nc.vector.memset(zerot, 0.0)
            nc.vector.memset(epst, EPS)
            identf = P.tile([128, 128], f32, name="identf")
            make_identity(nc, identf)
            nc.vector.tensor_copy(identb, identf)
            # ones row of v lhsT (value 32); zero the M-padding rows 65..127
            nc.vector.memset(v_allF[:, :, :, :, DH : DH + 1], 32.0)
            nc.vector.memset(v_allF[:, :, :, :, DH + 1 : 128], 0.0)

            with (
                tc.tile_pool(name="psCD", bufs=1, space="PSUM") as pP,
                tc.tile_pool(name="sbCD", bufs=1) as sD,
            ):
                # ---------- phase A: q/k rope projections + v ----------
                def rope_proj(dst, w_pre, w_shf, chunks, tag):
                    for cc in range(2):
                        for (ofs, W_) in chunks:
                            sl = slice(ofs, ofs + W_)
                            pre = pP.tile([128, 512], f32, tag="sc", bufs=2,
                                          name=f"pre{tag}{cc}{ofs}")
                            mm(pre[:, 0:W_], w_pre[:, :, cc * 128 : (cc + 1) * 128],
                               xF[:, :, sl], start=True, stop=True, perf_mode=DR)
                            shf = pP.tile([128, 512], f32, tag="sc", bufs=2,
                                          name=f"shf{tag}{cc}{ofs}")
                            mm(shf[:, 0:W_], w_shf[:, :, cc * 128 : (cc + 1) * 128],
                               xF[:, :, sl], start=True, stop=True, perf_mode=DR)
                            t1 = sD.tile([128, 512], f32, tag="t1", bufs=2, name="t1")
                            t2 = sD.tile([128, 512], f32, tag="t2", bufs=2, name="t2")
                            nc.vector.tensor_mul(t1[:, 0:W_], pre[:, 0:W_], ct[:, sl])
                            nc.vector.tensor_mul(t2[:, 0:W_], shf[:, 0:W_], st[:, sl])
                            nc.gpsimd.tensor_add(dst[cc][:, sl],
                                                 t1[:, 0:W_], t2[:, 0:W_])

                QCH = [(0, 512), (512, 512), (1024, 128)]
                KCH = [(o, min(512, N - o)) for o in range(0, N, 512)]
                rope_proj(qrot, wq, wqs, QCH[:1], "q0")
                rope_proj(krot, wk, wks, KCH, "k")
                for m in range(MT):
                    g, sv = divmod(m, 2)
                    psv = pP.tile([128, 256], f32, tag="sc", bufs=2, name=f"psv{m}")
                    mm(psv, xF[:, :, m * 128 : (m + 1) * 128], wv,
                       start=True, stop=True, perf_mode=DR)
                    nc.vector.tensor_copy(
                        v_allF[:, g, sv, :, 0:DH],
                        psv.rearrange("p (h d) -> p h d", h=NH),
                    )
                rope_proj(qrot, wq, wqs, QCH[1:], "q1")

                # ---------- per-j: attention, proj+LN1, FFN, LN2, out ----------
                def attn_head(j, h):
                    ofs, W_ = JW[j]
                    jsl = slice(ofs, ofs + W_)
                    hc, hp = h // 2, slice((h % 2) * 64, (h % 2) * 64 + 64)
                    at = pP.tile([128, 512], f32, tag="at", bufs=2, name=f"at{j}{h}")
                    for g in range(MG2):
                        scp = pP.tile([128, 2, 512], f32, tag="scp", bufs=2,
                                      name=f"sc{j}{h}{g}")
                        for si in range(2):
                            m = 2 * g + si
                            mm(scp[:, si, 0:W_],
                               krot[hc][hp, m * 128 : (m + 1) * 128],
                               qrot[hc][hp, jsl], start=True, stop=True)
                        exf = sD.tile([128, 2, 512], fp8, tag="ex", bufs=6,
                                      name=f"ex{j}{h}{g}")
                        if g in EXP_DVE_G:
                            nc.vector.tensor_scalar(
                                exf.bitcast(u8)[:, :, 0:W_], scp[:, :, 0:W_],
                                LOG2E8 / 8192.0, EXPBIAS, Alu.mult, Alu.add)
                        else:
                            nc.scalar.activation(exf[:, :, 0:W_], scp[:, :, 0:W_],
                                                 Act.Exp, scale=1.0 / 8192.0,
                                                 bias=ln8t[:, :])
                        mm(at[:, 0:W_], v_allF[:, g, :, h, :], exf[:, :, 0:W_],
                           start=(g == 0), stop=(g == MG2 - 1), perf_mode=DR)
                    denr = sD.tile([1, 512], f32, tag="row", bufs=12,
                                   name=f"denr{j}{h}")
                    nc.vector.tensor_copy(denr[:, 0:W_], at[DH : DH + 1, 0:W_])
                    den = sD.tile([1, 512], f32, tag="row", bufs=12, name=f"den{j}{h}")
                    nc.vector.reciprocal_approx_fast(den[:, 0:W_], denr[:, 0:W_])
                    rb = sD.tile([64, 512], f32, tag="rb", bufs=2, name=f"rb{j}{h}")
                    nc.gpsimd.partition_broadcast(rb[:, 0:W_], den[0:1, 0:W_])
                    nc.vector.scalar_tensor_tensor(
                        attnF[(h % 2) * 64 : (h % 2) * 64 + 64, h // 2, jsl],
                        at[0:DH, 0:W_], 32.0, rb[:, 0:W_], Alu.mult, Alu.mult)

                def ln_rows(pair, W_, tagp):
                    """-> (sum_b, rs): raw column-sum broadcast and 1/std bcast."""
                    pssum = pP.tile([1, 512], f32, tag="sc", bufs=2,
                                    name=f"pssum{tagp}")
                    for co in range(2):
                        mm(pssum[:, 0:W_], ones, pair[co],
                           start=(co == 0), stop=(co == 1))
                    pssq = pP.tile([1, 512], f32, tag="sc", bufs=2,
                                   name=f"pssq{tagp}")
                    for co in range(2):
                        sq = sD.tile([128, 512], f32r, tag="sq", bufs=2,
                                     name=f"sq{tagp}{co}")
                        nc.vector.tensor_mul(sq[:, 0:W_], pair[co], pair[co])
                        mm(pssq[:, 0:W_], ones, sq[:, 0:W_],
                           start=(co == 0), stop=(co == 1))
                    sumr = sD.tile([1, 512], f32, tag="row", bufs=12,
                                   name=f"sumr{tagp}")
                    nc.vector.tensor_copy(sumr[:, 0:W_], pssum[:, 0:W_])
                    u = sD.tile([1, 512], f32, tag="row", bufs=12, name=f"u{tagp}")
                    nc.vector.scalar_tensor_tensor(u[:, 0:W_], sumr[:, 0:W_], 1.0 / C,
                                                   sumr[:, 0:W_], Alu.mult, Alu.mult)
                    w_ = sD.tile([1, 512], f32, tag="row", bufs=12, name=f"w{tagp}")
                    nc.vector.tensor_sub(w_[:, 0:W_], pssq[:, 0:W_], u[:, 0:W_])
                    # rstd = 1/sqrt(var+eps) on DVE: magic-seed + 1 Newton step
                    i32 = mybir.dt.int32
                    vv = sD.tile([1, 512], f32, tag="row", bufs=12, name=f"vv{tagp}")
                    nc.vector.tensor_scalar(vv[:, 0:W_], w_[:, 0:W_], 1.0 / C, EPS,
                                            Alu.mult, Alu.add)
                    hi = sD.tile([1, 512], f32, tag="row", bufs=12,
                                 name=f"hi{tagp}")
                    nc.vector.tensor_scalar(hi.bitcast(i32)[:, 0:W_],
                                            vv.bitcast(i32)[:, 0:W_],
                                            1, None, Alu.arith_shift_right)
                    y0i = sD.tile([1, 512], f32, tag="row", bufs=12,
                                  name=f"y0i{tagp}")
                    nc.vector.tensor_scalar(y0i.bitcast(i32)[:, 0:W_],
                                            hi.bitcast(i32)[:, 0:W_],
                                            -1, 0x5F3759DF, Alu.mult, Alu.add)
                    t_ = sD.tile([1, 512], f32, tag="row", bufs=12, name=f"t{tagp}")
                    nc.vector.tensor_mul(t_[:, 0:W_], y0i[:, 0:W_], y0i[:, 0:W_])
                    nc.vector.tensor_mul(t_[:, 0:W_], t_[:, 0:W_], vv[:, 0:W_])
                    nc.vector.tensor_scalar(t_[:, 0:W_], t_[:, 0:W_], -0.5, 1.5,
                                            Alu.mult, Alu.add)
                    rstd = sD.tile([1, 512], f32, tag="row", bufs=12,
                                   name=f"rstd{tagp}")
                    nc.vector.tensor_mul(rstd[:, 0:W_], y0i[:, 0:W_], t_[:, 0:W_])
                    sum_b = sD.tile([128, 512], f32, tag="bc", bufs=4,
                                    name=f"sumb{tagp}")
                    nc.gpsimd.partition_broadcast(sum_b[:, 0:W_], sumr[0:1, 0:W_])
                    rs = sD.tile([128, 512], f32, tag="bc", bufs=4, name=f"rs{tagp}")
                    nc.gpsimd.partition_broadcast(rs[:, 0:W_], rstd[0:1, 0:W_])
                    return sum_b, rs

                def d_proj_ln1(j):
                    ofs, W_ = JW[j]
                    jsl = slice(ofs, ofs + W_)
                    res = []
                    for co in range(2):
                        psp = pP.tile([128, 512], f32, tag="sc", bufs=2,
                                      name=f"psp{j}{co}")
                        mm(psp[:, 0:W_], wp[:, :, co * 128 : (co + 1) * 128],
                           attnF[:, :, jsl], start=True, stop=True, perf_mode=DR)
                        rt = sD.tile([128, 512], f32r, tag="res", bufs=4,
                                     name=f"res{j}{co}")
                        nc.vector.scalar_tensor_tensor(rt[:, 0:W_], psp[:, 0:W_],
                                                       1.0 / 1024.0, xT[co][:, jsl],
                                                       Alu.mult, Alu.add)
                        res.append(rt[:, 0:W_])
                    sum_b, rs = ln_rows(res, W_, f"a{j}")
                    zgs = []
                    for co in range(2):
                        A = sD.tile([128, 512], f32, tag="za", bufs=2,
                                    name=f"A{j}{co}")
                        nc.vector.scalar_tensor_tensor(A[:, 0:W_], sum_b[:, 0:W_],
                                                       -1.0 / C, res[co],
                                                       Alu.mult, Alu.add)
                        z = sD.tile([128, 512], bf16, tag="zg", bufs=4,
                                    name=f"zg{j}{co}")
                        nc.vector.scalar_tensor_tensor(z[:, 0:W_], A[:, 0:W_],
                                                       g1c[co], rs[:, 0:W_],
                                                       Alu.mult, Alu.mult)
                        nc.gpsimd.tensor_copy(zgF[:, co, jsl], z[:, 0:W_])
                        zgs.append(z[:, 0:W_])
                    return zgs

                def d_ffn1(j):
                    ofs, W_ = JW[j]
                    jsl = slice(ofs, ofs + W_)
                    hts = sD.tile([128, 8, 512], fp8, tag="hts", bufs=2,
                                  name=f"hts{j}")
                    for f in range(8):
                        psh = pP.tile([128, 512], f32, tag="sc", bufs=2,
                                      name=f"psh{j}{f}")
                        mm(psh[:, 0:W_], w1[:, :, f * 128 : (f + 1) * 128],
                           zgF[:, :, jsl], start=True, stop=True, perf_mode=DR)
                        nc.scalar.activation(hts[:, f, 0:W_], psh[:, 0:W_], Act.Gelu,
                                             bias=bf1c[:, f : f + 1], scale=1.0 / 32.0)
                    return hts

                def d_ffn2_ln2_out(j, zgs, hts):
                    ofs, W_ = JW[j]
                    x2l = []
                    for co in range(2):
                        psf = pP.tile([128, 512], f32, tag="sc", bufs=2,
                                      name=f"psf{j}{co}")
                        for g2 in range(4):
                            mm(psf[:, 0:W_], w2[:, g2, :, co * 128 : (co + 1) * 128],
                               hts[:, 2 * g2 : 2 * g2 + 2, 0:W_],
                               start=(g2 == 0), stop=(g2 == 3), perf_mode=DR)
                        x2t = sD.tile([128, 512], f32, tag="za", bufs=2,
                                      name=f"x2t{j}{co}")
                        nc.vector.tensor_scalar(x2t[:, 0:W_], psf[:, 0:W_],
                                                1.0 / 32.0, B2c[co],
                                                Alu.mult, Alu.add)
                        x2 = sD.tile([128, 512], f32r, tag="x2", bufs=4,
                                     name=f"x2_{j}{co}")
                        nc.vector.tensor_add(x2[:, 0:W_], x2t[:, 0:W_], zgs[co])
                        x2l.append(x2[:, 0:W_])
                    sum_b2, rs2 = ln_rows(x2l, W_, f"b{j}")
                    for co in range(2):
                        A2 = sD.tile([128, 512], f32, tag="za", bufs=2,
                                     name=f"A2_{j}{co}")
                        nc.vector.scalar_tensor_tensor(A2[:, 0:W_], sum_b2[:, 0:W_],
                                                       -1.0 / C, x2l[co],
                                                       Alu.mult, Alu.add)
                        fz = sD.tile([128, 512], bf16, tag="fz", bufs=2,
                                     name=f"fz{j}{co}")
                        nc.vector.scalar_tensor_tensor(fz[:, 0:W_], A2[:, 0:W_],
                                                       g2c[co], rs2[:, 0:W_],
                                                       Alu.mult, Alu.mult)
                        if with_b2:
                            fo = sD.tile([128, 512], bf16, tag="fz", bufs=2,
                                         name=f"fo{j}{co}")
                            nc.gpsimd.tensor_scalar_add(fo[:, 0:W_], fz[:, 0:W_],
                                                        b2c[co])
                            fz = fo
                        x2l[co] = fz
                    for tt in range(W_ // 128):
                        t0 = ofs + tt * 128
                        tok = sD.tile([128, C], bf16, tag="tok", bufs=3,
                                      name=f"tok{j}{tt}")
                        for co in range(2):
                            pst = pP.tile([128, 128], bf16, tag="sc", bufs=2,
                                          name=f"pst{j}{tt}{co}")
                            nc.tensor.transpose(
                                pst, x2l[co][:, tt * 128 : (tt + 1) * 128], identb
                            )
                            csl = slice(co * 128, (co + 1) * 128)
                            if co == 0:
                                nc.vector.tensor_copy(tok[:, csl], pst)
                            else:
                                nc.scalar.copy(tok[:, csl], pst)
                        nc.sync.dma_start(d_out[t0 : t0 + 128, :], tok)

                for j in range(NJ):
                    for h in range(NH):
                        attn_head(j, h)
                    zgs = d_proj_ln1(j)
                    hts = d_ffn1(j)
                    d_ffn2_ln2_out(j, zgs, hts)

                if debug:
                    for i in range(2):
                        nc.sync.dma_start(d_dbg["dbg_qrot%d" % i][:, :], qrot[i])
                        nc.sync.dma_start(d_dbg["dbg_krot%d" % i][:, :], krot[i])
                    nc.sync.dma_start(d_dbg["dbg_v"][:, :, :, :, :], v_allF)
                    nc.sync.dma_start(d_dbg["dbg_attnF"][:, :, :], attnF)
                    nc.sync.dma_start(d_dbg["dbg_zgF"][:, :, :], zgF)

    nc.compile()
    return nc


def _get_program(with_b2, debug=False):
    key = f"nc{int(with_b2)}_{int(debug)}"
    if key not in _CACHE:
        _CACHE[key] = _build_program(with_b2, debug)
    return _CACHE[key]


def _host_prep(x, Wqkv, Wproj, g1, b1, g2, b2, W1, bf1, W2, bf2, H, W):
    import ml_dtypes

    f8 = ml_dtypes.float8_e4m3
    bf = ml_dtypes.bfloat16
    f32 = np.float32

    Wq, Wk, Wv = Wqkv[0:C], Wqkv[C : 2 * C], Wqkv[2 * C : 3 * C]

    # rotate-half partner: pair-swap within each 32-dim rope half (d ^ 1)
    permC = np.arange(C) ^ 1

    def proj_tiles(Wm):
        """-> (pre, shf) [128, 2, 256] fp8, x32, out cols in original order."""
        pre = (Wm * 32.0).T.reshape(2, 128, 256).transpose(1, 0, 2)
        shf = (Wm[permC] * 32.0).T.reshape(2, 128, 256).transpose(1, 0, 2)
        return (np.ascontiguousarray(pre).astype(f8),
                np.ascontiguousarray(shf).astype(f8))

    wq8, wqs8 = proj_tiles(Wq)
    wk8, wks8 = proj_tiles(Wk)
    wv8 = np.ascontiguousarray((Wv * 32.0).T.reshape(2, 128, 256)
                               .transpose(1, 0, 2)).astype(f8)
    wp8 = np.ascontiguousarray((Wproj * 32.0).T.reshape(2, 128, 256)
                               .transpose(1, 0, 2)).astype(f8)
    w18 = np.ascontiguousarray((W1 * 32.0).T.reshape(2, 128, F)
                               .transpose(1, 0, 2)).astype(f8)
    # w2 [128, 4, 2, 256]: in-feature (g2, s, p) = (2*g2+s)*128+p
    w28 = np.ascontiguousarray((W2 * 32.0).T.reshape(4, 2, 128, C)
                               .transpose(2, 0, 1, 3)).astype(f8)

    shared = {
        "wq": wq8, "wqs": wqs8, "wk": wk8, "wks": wks8, "wv": wv8, "wp": wp8,
        "w1": w18, "w2": w28,
        "g1c": np.ascontiguousarray(g1.reshape(C, 1), dtype=f32),
        "g2c": np.ascontiguousarray(g2.reshape(C, 1), dtype=f32),
        "B2c": np.ascontiguousarray((b1 + bf2).reshape(C, 1), dtype=f32),
        "b2c": np.ascontiguousarray(b2.reshape(C, 1), dtype=f32),
        "bf1c": np.ascontiguousarray((bf1 + W1 @ b1).reshape(F, 1), dtype=f32),
    }

    # rope tables, baseline row layout: row d (of 64-dim head block) x 2
    invf = 1.0 / (10000.0 ** (np.arange(HALF, dtype=np.float64) / HALF))
    yy, xx = np.meshgrid(np.arange(H), np.arange(W), indexing="ij")
    pos_y = yy.reshape(-1).astype(np.float64)
    pos_x = xx.reshape(-1).astype(np.float64)
    ang = np.concatenate(
        [invf[:, None] * pos_y[None, :], invf[:, None] * pos_x[None, :]], axis=0
    )  # [64, N]
    sgn = np.where(np.arange(DH) % 2 == 0, -1.0, 1.0)[:, None]
    ct64 = np.cos(ang)
    st64 = np.sin(ang) * sgn
    ct128 = np.concatenate([ct64, ct64], axis=0)  # [128, N]
    st128 = np.concatenate([st64, st64], axis=0)

    in_maps = []
    for core in range(NCORES):
        b, qh = core // 2, core % 2
        n0 = qh * NQ
        rot = np.concatenate([np.arange(n0, N), np.arange(0, n0)])
        m = dict(shared)
        xb = x[b][rot]                                          # [N, C]
        m["xT"] = np.ascontiguousarray(xb.T, dtype=f32)
        m["xF"] = np.ascontiguousarray(
            xb.T.reshape(2, 128, N).transpose(1, 0, 2)).astype(f8)
        m["ct"] = np.ascontiguousarray(ct128[:, rot]).astype(bf)
        m["st"] = np.ascontiguousarray(st128[:, rot]).astype(bf)
        in_maps.append(m)
    return in_maps


def kernel(x, Wqkv, Wproj, g1, b1, g2, b2, W1, bf1, W2, bf2, H, W, **kw):
    from concourse.bass_utils import run_bass_kernel_spmd

    x = np.asarray(x, dtype=np.float32)
    args = [np.asarray(a, dtype=np.float32)
            for a in (Wqkv, Wproj, g1, b1, g2, b2, W1, bf1, W2, bf2)]
    H, W = int(H), int(W)

    with_b2 = bool(np.any(args[5]))
    nc = _get_program(with_b2, _CACHE.get("debug", False))
    in_maps = _host_prep(x, *args, H, W)
    res = run_bass_kernel_spmd(nc, in_maps, core_ids=list(range(NCORES)),
                               **_CACHE.get("run_kwargs", {}))
    _CACHE["last_result"] = res

    out = np.zeros((B, N, C), dtype=np.float32)
    for core in range(NCORES):
        b, qh = core // 2, core % 2
        n0 = qh * NQ
        out[b, n0 : n0 + NQ, :] = res.results[core]["out"].astype(np.float32)
    return out


# revision 14
# speedup vs baseline: 1.4678x; 1.1754x over previous
"""Self-contained Trainium2 Bass kernel for the AttnBlock problem — fp8 edition.

Sharding: 8 cores; core c handles batch b = c//2, query rows
[qh*1152, (qh+1)*1152) with qh = c%2.  Each core computes full K/V for its
batch (duplicated across the 2 cores of a batch) so there are NO collectives.
Per-core token order is rotated so each core's own query tokens are always
columns 0..1151.

All heavy matmuls run in fp8e4m3 with DoubleRow perf mode (2 contraction
subtiles per pass, 0.5 cycles/column).  Weights are host-scaled by 32 into
the fp8 sweet spot; activations carry powers-of-two scales that are folded
into downstream scalars:
  qrot/krot = bf16(32*rope(q|k)) (scores stay bf16 — contraction is 64, so
  fp8 wins nothing there), scores PSUM = 8192*s (s = q.k/sqrt(64))
  exp: 8*exp(s) as fp8 — on ACT (Exp table, scale 1/8192, bias ln8) or on
  DVE via the direct fp8-byte trick u8 = round(8*log2(e)*s + 80).
  v_allF = fp8(32*v) with a 32-valued ones row at index 64 (lhsT padded to
  M=128; rows 65..127 are never read downstream).
  attnF = fp8(32*attn_out); proj PSUM = 1024*proj.
  zgF = fp8(LN1 out); FFN1 PSUM = 32*h; hts = fp8(gelu(h));
  FFN2 PSUM = 32*(ffn2 + zg) via an extra 32*identity bf16 matmul.

LayerNorm rstd = exp(-0.5*ln(var+eps)) — Ln and Exp share one ACT table
with the attention exps, so only Gelu causes table reloads.
"""

import numpy as np

B, N, C = 4, 2304, 256
NH, DH = 4, 64
HALF = DH // 2         # 32 (rope half)
NQ = N // 2            # 1152
F = 4 * C              # 1024
NCORES = 8
MT = N // 128          # 18 key tiles
MG2 = MT // 2          # 9 key-tile pairs
EPS = 1e-5
LOG2E8 = 11.541560327111707   # 8*log2(e)
EXPBIAS = 80.0                # (7+3)*8: fp8 byte bias for 8*exp(s)
JW = [(0, 384), (384, 384), (768, 384)]
NJ = 3
EXP_DVE_G = (4,)       # key-tile-pair indices whose exp runs on DVE

_CACHE = {}


def _build_program(with_b2, debug=False):
    import concourse.tile as tile
    from concourse import bacc, mybir
    from concourse.masks import make_identity

    f32 = mybir.dt.float32
    f32r = mybir.dt.float32r
    bf16 = mybir.dt.bfloat16
    fp8 = mybir.dt.float8e4
    u8 = mybir.dt.uint8
    Alu = mybir.AluOpType
    Act = mybir.ActivationFunctionType
    DR = mybir.MatmulPerfMode.DoubleRow

    nc = bacc.Bacc(None, target_bir_lowering=False, debug=False)

    def dram(name, shape, dt=f32, out=False):
        return nc.dram_tensor(
            name, list(shape), dt, kind="ExternalOutput" if out else "ExternalInput"
        )

    d_xT = dram("xT", [C, N])
    d_xF = dram("xF", [128, 2, N], fp8)
    d_ct = dram("ct", [128, N], bf16)
    d_st = dram("st", [128, N], bf16)
    d_wq = dram("wq", [128, 2, 256], fp8)
    d_wqs = dram("wqs", [128, 2, 256], fp8)
    d_wk = dram("wk", [128, 2, 256], fp8)
    d_wks = dram("wks", [128, 2, 256], fp8)
    d_wv = dram("wv", [128, 2, 256], fp8)
    d_wp = dram("wp", [128, 2, 256], fp8)
    d_w1 = dram("w1", [128, 2, F], fp8)
    d_w2 = dram("w2", [128, 4, 2, 256], fp8)
    d_g1 = dram("g1c", [C, 1])
    d_g2 = dram("g2c", [C, 1])
    d_B2 = dram("B2c", [C, 1])
    d_b2 = dram("b2c", [C, 1])
    d_bf1 = dram("bf1c", [F, 1])
    d_out = nc.dram_tensor("out", [NQ, C], bf16, kind="ExternalOutput")
    if debug:
        d_dbg = {
            "dbg_qrot0": nc.dram_tensor("dbg_qrot0", [128, NQ], bf16, kind="ExternalOutput"),
            "dbg_qrot1": nc.dram_tensor("dbg_qrot1", [128, NQ], bf16, kind="ExternalOutput"),
            "dbg_krot0": nc.dram_tensor("dbg_krot0", [128, N], bf16, kind="ExternalOutput"),
            "dbg_krot1": nc.dram_tensor("dbg_krot1", [128, N], bf16, kind="ExternalOutput"),
            "dbg_v": dram("dbg_v", [128, MG2, 2, NH, 128], fp8, out=True),
            "dbg_attnF": dram("dbg_attnF", [128, 2, NQ], fp8, out=True),
            "dbg_zgF": dram("dbg_zgF", [128, 2, NQ], fp8, out=True),
        }

    mm = nc.tensor.matmul

    with tile.TileContext(nc) as tc:
        with tc.tile_pool(name="persist", bufs=1) as P:
            xT = [P.tile([128, N], f32, name=f"xT{i}") for i in range(2)]
            xF = P.tile([128, 2, N], fp8, name="xF")
            ct = P.tile([128, N], bf16, name="ct")
            st = P.tile([128, N], bf16, name="st")
            wq = P.tile([128, 2, 256], fp8, name="wq")
            wqs = P.tile([128, 2, 256], fp8, name="wqs")
            wk = P.tile([128, 2, 256], fp8, name="wk")
            wks = P.tile([128, 2, 256], fp8, name="wks")
            wv = P.tile([128, 2, 256], fp8, name="wv")
            wp = P.tile([128, 2, 256], fp8, name="wp")
            w1 = P.tile([128, 2, F], fp8, name="w1")
            w2 = P.tile([128, 4, 2, 256], fp8, name="w2")
            g1c = [P.tile([128, 1], f32, name=f"g1c{i}") for i in range(2)]
            g2c = [P.tile([128, 1], f32, name=f"g2c{i}") for i in range(2)]
            B2c = [P.tile([128, 1], f32, name=f"B2c{i}") for i in range(2)]
            b2c = [P.tile([128, 1], f32, name=f"b2c{i}") for i in range(2)]
            bf1c = P.tile([128, 8], f32, name="bf1c")
            ln8t = P.tile([128, 1], f32, name="ln8t")
            zerot = P.tile([128, 1], f32, name="zerot")
            epst = P.tile([128, 1], f32, name="epst")
            identb = P.tile([128, 128], bf16, name="identb")
            ones = P.tile([128, 1], f32r, name="ones")
            qrot = [P.tile([128, NQ], bf16, name=f"qrot{i}") for i in range(2)]
            krot = [P.tile([128, N], bf16, name=f"krot{i}") for i in range(2)]
            v_allF = P.tile([128, MG2, 2, NH, 128], fp8, name="v_allF")
            attnF = P.tile([128, 2, NQ], fp8, name="attnF")
            zgF = P.tile([128, 2, NQ], fp8, name="zgF")

            # critical-path loads first
            nc.sync.dma_start(xF, d_xF[:, :, :])
            nc.sync.dma_start(ct, d_ct[:, :])
            nc.sync.dma_start(st, d_st[:, :])
            for t, d in [(wq, d_wq), (wqs, d_wqs), (wk, d_wk), (wks, d_wks),
                         (wv, d_wv)]:
                nc.sync.dma_start(t, d[:, :, :])
            nc.sync.dma_start(wp, d_wp[:, :, :])
            nc.sync.dma_start(w1, d_w1[:, :, :])
            nc.sync.dma_start(w2, d_w2[:, :, :, :])
            for i in range(2):
                nc.sync.dma_start(g1c[i], d_g1[i * 128 : (i + 1) * 128, :])
                nc.sync.dma_start(g2c[i], d_g2[i * 128 : (i + 1) * 128, :])
                nc.sync.dma_start(B2c[i], d_B2[i * 128 : (i + 1) * 128, :])
                nc.sync.dma_start(b2c[i], d_b2[i * 128 : (i + 1) * 128, :])
                nc.sync.dma_start(xT[i], d_xT[i * 128 : (i + 1) * 128, :])
            for i in range(8):
                nc.sync.dma_start(bf1c[:, i : i + 1], d_bf1[i * 128 : (i + 1) * 128, :])

            onesf = P.tile([128, 1], f32, name="onesf")
            nc.vector.memset(onesf, 1.0)
            nc.vector.tensor_copy(ones, onesf)
            nc.vector.memset(ln8t, float(np.log(8.0)))
            nc.vector.memset(zerot, 0.0)
            nc.vector.memset(epst, EPS)
            identf = P.tile([128, 128], f32, name="identf")
            make_identity(nc, identf)
            nc.vector.tensor_copy(identb, identf)
            # ones row of v lhsT (value 32); zero the M-padding rows 65..127
            nc.vector.memset(v_allF[:, :, :, :, DH : DH + 1], 32.0)
            nc.vector.memset(v_allF[:, :, :, :, DH + 1 : 128], 0.0)

            with (
                tc.tile_pool(name="psCD", bufs=1, space="PSUM") as pP,
                tc.tile_pool(name="sbCD", bufs=1) as sD,
            ):
                # ---------- phase A: q/k rope projections + v ----------
                def rope_proj(dst, w_pre, w_shf, chunks, tag):
                    for cc in range(2):
                        for (ofs, W_) in chunks:
                            sl = slice(ofs, ofs + W_)
                            pre = pP.tile([128, 512], f32, tag="sc", bufs=2,
                                          name=f"pre{tag}{cc}{ofs}")
                            mm(pre[:, 0:W_], w_pre[:, :, cc * 128 : (cc + 1) * 128],
                               xF[:, :, sl], start=True, stop=True, perf_mode=DR)
                            shf = pP.tile([128, 512], f32, tag="sc", bufs=2,
                                          name=f"shf{tag}{cc}{ofs}")
                            mm(shf[:, 0:W_], w_shf[:, :, cc * 128 : (cc + 1) * 128],
                               xF[:, :, sl], start=True, stop=True, perf_mode=DR)
                            t1 = sD.tile([128, 512], f32, tag="t1", bufs=2, name="t1")
                            t2 = sD.tile([128, 512], f32, tag="t2", bufs=2, name="t2")
                            nc.vector.tensor_mul(t1[:, 0:W_], pre[:, 0:W_], ct[:, sl])
                            nc.vector.tensor_mul(t2[:, 0:W_], shf[:, 0:W_], st[:, sl])
                            nc.gpsimd.tensor_add(dst[cc][:, sl],
                                                 t1[:, 0:W_], t2[:, 0:W_])

                QCH = [(0, 384), (384, 384), (768, 384)]
                KCH = [(o, min(512, N - o)) for o in range(0, N, 512)]
                rope_proj(qrot, wq, wqs, QCH[:1], "q0")
                rope_proj(krot, wk, wks, KCH, "k")
                for m in range(MT):
                    g, sv = divmod(m, 2)
                    psv = pP.tile([128, 256], f32, tag="sc", bufs=2, name=f"psv{m}")
                    mm(psv, xF[:, :, m * 128 : (m + 1) * 128], wv,
                       start=True, stop=True, perf_mode=DR)
                    nc.scalar.copy(
                        v_allF[:, g, sv, :, 0:DH],
                        psv.rearrange("p (h d) -> p h d", h=NH),
                    )
                rope_proj(qrot, wq, wqs, QCH[1:], "q1")

                # ---------- per-j: attention, proj+LN1, FFN, LN2, out ----------
                def attn_head(j, h):
                    ofs, W_ = JW[j]
                    jsl = slice(ofs, ofs + W_)
                    hc, hp = h // 2, slice((h % 2) * 64, (h % 2) * 64 + 64)
                    at = pP.tile([128, 512], f32, tag="at", bufs=2, name=f"at{j}{h}")
                    for g in range(MG2):
                        scp = pP.tile([128, 2, 512], f32, tag="scp", bufs=2,
                                      name=f"sc{j}{h}{g}")
                        for si in range(2):
                            m = 2 * g + si
                            mm(scp[:, si, 0:W_],
                               krot[hc][hp, m * 128 : (m + 1) * 128],
                               qrot[hc][hp, jsl], start=True, stop=True)
                        exf = sD.tile([128, 2, 512], fp8, tag="ex", bufs=6,
                                      name=f"ex{j}{h}{g}")
                        if g in EXP_DVE_G:
                            nc.vector.tensor_scalar(
                                exf.bitcast(u8)[:, :, 0:W_], scp[:, :, 0:W_],
                                LOG2E8 / 8192.0, EXPBIAS, Alu.mult, Alu.add)
                        else:
                            nc.scalar.activation(exf[:, :, 0:W_], scp[:, :, 0:W_],
                                                 Act.Exp, scale=1.0 / 8192.0,
                                                 bias=ln8t[:, :])
                        mm(at[:, 0:W_], v_allF[:, g, :, h, :], exf[:, :, 0:W_],
                           start=(g == 0), stop=(g == MG2 - 1), perf_mode=DR)
                    denr = sD.tile([1, 512], f32, tag="row", bufs=12,
                                   name=f"denr{j}{h}")
                    nc.vector.tensor_copy(denr[:, 0:W_], at[DH : DH + 1, 0:W_])
                    den = sD.tile([1, 512], f32, tag="row", bufs=12, name=f"den{j}{h}")
                    nc.vector.reciprocal_approx_fast(den[:, 0:W_], denr[:, 0:W_])
                    rb = sD.tile([64, 512], f32, tag="rb", bufs=2, name=f"rb{j}{h}")
                    nc.gpsimd.partition_broadcast(rb[:, 0:W_], den[0:1, 0:W_])
                    nc.vector.scalar_tensor_tensor(
                        attnF[(h % 2) * 64 : (h % 2) * 64 + 64, h // 2, jsl],
                        at[0:DH, 0:W_], 32.0, rb[:, 0:W_], Alu.mult, Alu.mult)

                def ln_rows(pair, W_, tagp):
                    """-> (sum_b, rs): raw column-sum broadcast and 1/std bcast."""
                    pssum = pP.tile([1, 512], f32, tag="sc", bufs=2,
                                    name=f"pssum{tagp}")
                    for co in range(2):
                        mm(pssum[:, 0:W_], ones, pair[co],
                           start=(co == 0), stop=(co == 1))
                    pssq = pP.tile([1, 512], f32, tag="sc", bufs=2,
                                   name=f"pssq{tagp}")
                    for co in range(2):
                        sq = sD.tile([128, 512], f32r, tag="sq", bufs=2,
                                     name=f"sq{tagp}{co}")
                        nc.vector.tensor_mul(sq[:, 0:W_], pair[co], pair[co])
                        mm(pssq[:, 0:W_], ones, sq[:, 0:W_],
                           start=(co == 0), stop=(co == 1))
                    sumr = sD.tile([1, 512], f32, tag="row", bufs=12,
                                   name=f"sumr{tagp}")
                    nc.vector.tensor_copy(sumr[:, 0:W_], pssum[:, 0:W_])
                    # var = pssq/C - (sum/C)^2 (+eps folded away; var >> eps here)
                    u = sD.tile([1, 512], f32, tag="row", bufs=12, name=f"u{tagp}")
                    nc.vector.scalar_tensor_tensor(u[:, 0:W_], sumr[:, 0:W_],
                                                   -1.0 / (C * C),
                                                   sumr[:, 0:W_], Alu.mult, Alu.mult)
                    i32 = mybir.dt.int32
                    vv = sD.tile([1, 512], f32, tag="row", bufs=12, name=f"vv{tagp}")
                    nc.vector.scalar_tensor_tensor(vv[:, 0:W_], pssq[:, 0:W_], 1.0 / C,
                                                   u[:, 0:W_], Alu.mult, Alu.add)
                    hi = sD.tile([1, 512], f32, tag="row", bufs=12,
                                 name=f"hi{tagp}")
                    nc.vector.tensor_scalar(hi.bitcast(i32)[:, 0:W_],
                                            vv.bitcast(i32)[:, 0:W_],
                                            1, None, Alu.arith_shift_right)
                    y0i = sD.tile([1, 512], f32, tag="row", bufs=12,
                                  name=f"y0i{tagp}")
                    nc.vector.tensor_scalar(y0i.bitcast(i32)[:, 0:W_],
                                            hi.bitcast(i32)[:, 0:W_],
                                            -1, 0x5F3759DF, Alu.mult, Alu.add)
                    t_ = sD.tile([1, 512], f32, tag="row", bufs=12, name=f"t{tagp}")
                    nc.vector.tensor_mul(t_[:, 0:W_], y0i[:, 0:W_], y0i[:, 0:W_])
                    nc.vector.tensor_mul(t_[:, 0:W_], t_[:, 0:W_], vv[:, 0:W_])
                    nc.vector.tensor_scalar(t_[:, 0:W_], t_[:, 0:W_], -0.5, 1.5,
                                            Alu.mult, Alu.add)
                    rstd = sD.tile([1, 512], f32, tag="row", bufs=12,
                                   name=f"rstd{tagp}")
                    nc.vector.tensor_mul(rstd[:, 0:W_], y0i[:, 0:W_], t_[:, 0:W_])
                    sum_b = sD.tile([128, 512], f32, tag="bc", bufs=4,
                                    name=f"sumb{tagp}")
                    nc.gpsimd.partition_broadcast(sum_b[:, 0:W_], sumr[0:1, 0:W_])
                    rs = sD.tile([128, 512], f32, tag="bc", bufs=4, name=f"rs{tagp}")
                    nc.gpsimd.partition_broadcast(rs[:, 0:W_], rstd[0:1, 0:W_])
                    return sum_b, rs

                def d_proj_ln1(j):
                    ofs, W_ = JW[j]
                    jsl = slice(ofs, ofs + W_)
                    res = []
                    for co in range(2):
                        psp = pP.tile([128, 512], f32, tag="sc", bufs=2,
                                      name=f"psp{j}{co}")
                        mm(psp[:, 0:W_], wp[:, :, co * 128 : (co + 1) * 128],
                           attnF[:, :, jsl], start=True, stop=True, perf_mode=DR)
                        rt = sD.tile([128, 512], f32r, tag="res", bufs=4,
                                     name=f"res{j}{co}")
                        nc.vector.scalar_tensor_tensor(rt[:, 0:W_], psp[:, 0:W_],
                                                       1.0 / 1024.0, xT[co][:, jsl],
                                                       Alu.mult, Alu.add)
                        res.append(rt[:, 0:W_])
                    sum_b, rs = ln_rows(res, W_, f"a{j}")
                    zgs = []
                    for co in range(2):
                        A = sD.tile([128, 512], f32, tag="za", bufs=2,
                                    name=f"A{j}{co}")
                        nc.vector.scalar_tensor_tensor(A[:, 0:W_], sum_b[:, 0:W_],
                                                       -1.0 / C, res[co],
                                                       Alu.mult, Alu.add)
                        z = sD.tile([128, 512], bf16, tag="zg", bufs=4,
                                    name=f"zg{j}{co}")
                        nc.vector.scalar_tensor_tensor(z[:, 0:W_], A[:, 0:W_],
                                                       g1c[co], rs[:, 0:W_],
                                                       Alu.mult, Alu.mult)
                        nc.gpsimd.tensor_copy(zgF[:, co, jsl], z[:, 0:W_])
                        zgs.append(z[:, 0:W_])
                    return zgs

                def d_ffn1(j):
                    ofs, W_ = JW[j]
                    jsl = slice(ofs, ofs + W_)
                    hts = sD.tile([128, 8, 512], fp8, tag="hts", bufs=2,
                                  name=f"hts{j}")
                    for f in range(8):
                        psh = pP.tile([128, 512], f32, tag="sc", bufs=2,
                                      name=f"psh{j}{f}")
                        mm(psh[:, 0:W_], w1[:, :, f * 128 : (f + 1) * 128],
                           zgF[:, :, jsl], start=True, stop=True, perf_mode=DR)
                        nc.scalar.activation(hts[:, f, 0:W_], psh[:, 0:W_], Act.Gelu,
                                             bias=bf1c[:, f : f + 1], scale=1.0 / 32.0)
                    return hts

                def d_ffn2_ln2_out(j, zgs, hts):
                    ofs, W_ = JW[j]
                    x2l = []
                    for co in range(2):
                        psf = pP.tile([128, 512], f32, tag="sc", bufs=2,
                                      name=f"psf{j}{co}")
                        for g2 in range(4):
                            mm(psf[:, 0:W_], w2[:, g2, :, co * 128 : (co + 1) * 128],
                               hts[:, 2 * g2 : 2 * g2 + 2, 0:W_],
                               start=(g2 == 0), stop=(g2 == 3), perf_mode=DR)
                        x2t = sD.tile([128, 512], f32, tag="za", bufs=2,
                                      name=f"x2t{j}{co}")
                        nc.vector.tensor_scalar(x2t[:, 0:W_], psf[:, 0:W_],
                                                1.0 / 32.0, B2c[co],
                                                Alu.mult, Alu.add)
                        x2 = sD.tile([128, 512], f32r, tag="x2", bufs=4,
                                     name=f"x2_{j}{co}")
                        nc.vector.tensor_add(x2[:, 0:W_], x2t[:, 0:W_], zgs[co])
                        x2l.append(x2[:, 0:W_])
                    sum_b2, rs2 = ln_rows(x2l, W_, f"b{j}")
                    for co in range(2):
                        A2 = sD.tile([128, 512], f32, tag="za", bufs=2,
                                     name=f"A2_{j}{co}")
                        nc.vector.scalar_tensor_tensor(A2[:, 0:W_], sum_b2[:, 0:W_],
                                                       -1.0 / C, x2l[co],
                                                       Alu.mult, Alu.add)
                        fz = sD.tile([128, 512], bf16, tag="fz", bufs=2,
                                     name=f"fz{j}{co}")
                        nc.vector.scalar_tensor_tensor(fz[:, 0:W_], A2[:, 0:W_],
                                                       g2c[co], rs2[:, 0:W_],
                                                       Alu.mult, Alu.mult)
                        if with_b2:
                            fo = sD.tile([128, 512], bf16, tag="fz", bufs=2,
                                         name=f"fo{j}{co}")
                            nc.gpsimd.tensor_scalar_add(fo[:, 0:W_], fz[:, 0:W_],
                                                        b2c[co])
                            fz = fo
                        x2l[co] = fz
                    for tt in range(W_ // 128):
                        t0 = ofs + tt * 128
                        tok = sD.tile([128, C], bf16, tag="tok", bufs=3,
                                      name=f"tok{j}{tt}")
                        for co in range(2):
                            pst = pP.tile([128, 128], bf16, tag="sc", bufs=2,
                                          name=f"pst{j}{tt}{co}")
                            nc.tensor.transpose(
                                pst, x2l[co][:, tt * 128 : (tt + 1) * 128], identb
                            )
                            csl = slice(co * 128, (co + 1) * 128)
                            if co == 0:
                                nc.vector.tensor_copy(tok[:, csl], pst)
                            else:
                                nc.scalar.copy(tok[:, csl], pst)
                        nc.sync.dma_start(d_out[t0 : t0 + 128, :], tok)

                for j in range(NJ):
                    for h in range(NH):
                        attn_head(j, h)
                    zgs = d_proj_ln1(j)
                    hts = d_ffn1(j)
                    d_ffn2_ln2_out(j, zgs, hts)

                if debug:
                    for i in range(2):
                        nc.sync.dma_start(d_dbg["dbg_qrot%d" % i][:, :], qrot[i])
                        nc.sync.dma_start(d_dbg["dbg_krot%d" % i][:, :], krot[i])
                    nc.sync.dma_start(d_dbg["dbg_v"][:, :, :, :, :], v_allF)
                    nc.sync.dma_start(d_dbg["dbg_attnF"][:, :, :], attnF)
                    nc.sync.dma_start(d_dbg["dbg_zgF"][:, :, :], zgF)

    nc.compile()
    return nc


def _get_program(with_b2, debug=False):
    key = f"nc{int(with_b2)}_{int(debug)}"
    if key not in _CACHE:
        _CACHE[key] = _build_program(with_b2, debug)
    return _CACHE[key]


def _host_prep(x, Wqkv, Wproj, g1, b1, g2, b2, W1, bf1, W2, bf2, H, W):
    import ml_dtypes

    f8 = ml_dtypes.float8_e4m3
    bf = ml_dtypes.bfloat16
    f32 = np.float32

    Wq, Wk, Wv = Wqkv[0:C], Wqkv[C : 2 * C], Wqkv[2 * C : 3 * C]

    # rotate-half partner: pair-swap within each 32-dim rope half (d ^ 1)
    permC = np.arange(C) ^ 1

    def proj_tiles(Wm):
        """-> (pre, shf) [128, 2, 256] fp8, x32, out cols in original order."""
        pre = (Wm * 32.0).T.reshape(2, 128, 256).transpose(1, 0, 2)
        shf = (Wm[permC] * 32.0).T.reshape(2, 128, 256).transpose(1, 0, 2)
        return (np.ascontiguousarray(pre).astype(f8),
                np.ascontiguousarray(shf).astype(f8))

    wq8, wqs8 = proj_tiles(Wq)
    wk8, wks8 = proj_tiles(Wk)
    wv8 = np.ascontiguousarray((Wv * 32.0).T.reshape(2, 128, 256)
                               .transpose(1, 0, 2)).astype(f8)
    wp8 = np.ascontiguousarray((Wproj * 32.0).T.reshape(2, 128, 256)
                               .transpose(1, 0, 2)).astype(f8)
    w18 = np.ascontiguousarray((W1 * 32.0).T.reshape(2, 128, F)
                               .transpose(1, 0, 2)).astype(f8)
    # w2 [128, 4, 2, 256]: in-feature (g2, s, p) = (2*g2+s)*128+p
    w28 = np.ascontiguousarray((W2 * 32.0).T.reshape(4, 2, 128, C)
                               .transpose(2, 0, 1, 3)).astype(f8)

    shared = {
        "wq": wq8, "wqs": wqs8, "wk": wk8, "wks": wks8, "wv": wv8, "wp": wp8,
        "w1": w18, "w2": w28,
        "g1c": np.ascontiguousarray(g1.reshape(C, 1), dtype=f32),
        "g2c": np.ascontiguousarray(g2.reshape(C, 1), dtype=f32),
        "B2c": np.ascontiguousarray((b1 + bf2).reshape(C, 1), dtype=f32),
        "b2c": np.ascontiguousarray(b2.reshape(C, 1), dtype=f32),
        "bf1c": np.ascontiguousarray((bf1 + W1 @ b1).reshape(F, 1), dtype=f32),
    }

    # rope tables, baseline row layout: row d (of 64-dim head block) x 2
    invf = 1.0 / (10000.0 ** (np.arange(HALF, dtype=np.float64) / HALF))
    yy, xx = np.meshgrid(np.arange(H), np.arange(W), indexing="ij")
    pos_y = yy.reshape(-1).astype(np.float64)
    pos_x = xx.reshape(-1).astype(np.float64)
    ang = np.concatenate(
        [invf[:, None] * pos_y[None, :], invf[:, None] * pos_x[None, :]], axis=0
    )  # [64, N]
    sgn = np.where(np.arange(DH) % 2 == 0, -1.0, 1.0)[:, None]
    ct64 = np.cos(ang)
    st64 = np.sin(ang) * sgn
    ct128 = np.concatenate([ct64, ct64], axis=0)  # [128, N]
    st128 = np.concatenate([st64, st64], axis=0)

    in_maps = []
    for core in range(NCORES):
        b, qh = core // 2, core % 2
        n0 = qh * NQ
        rot = np.concatenate([np.arange(n0, N), np.arange(0, n0)])
        m = dict(shared)
        xb = x[b][rot]                                          # [N, C]
        m["xT"] = np.ascontiguousarray(xb.T, dtype=f32)
        m["xF"] = np.ascontiguousarray(
            xb.T.reshape(2, 128, N).transpose(1, 0, 2)).astype(f8)
        m["ct"] = np.ascontiguousarray(ct128[:, rot]).astype(bf)
        m["st"] = np.ascontiguousarray(st128[:, rot]).astype(bf)
        in_maps.append(m)
    return in_maps


def kernel(x, Wqkv, Wproj, g1, b1, g2, b2, W1, bf1, W2, bf2, H, W, **kw):
    from concourse.bass_utils import run_bass_kernel_spmd

    x = np.asarray(x, dtype=np.float32)
    args = [np.asarray(a, dtype=np.float32)
            for a in (Wqkv, Wproj, g1, b1, g2, b2, W1, bf1, W2, bf2)]
    H, W = int(H), int(W)

    with_b2 = bool(np.any(args[5]))
    nc = _get_program(with_b2, _CACHE.get("debug", False))
    in_maps = _host_prep(x, *args, H, W)
    res = run_bass_kernel_spmd(nc, in_maps, core_ids=list(range(NCORES)),
                               **_CACHE.get("run_kwargs", {}))
    _CACHE["last_result"] = res

    out = np.zeros((B, N, C), dtype=np.float32)
    for core in range(NCORES):
        b, qh = core // 2, core % 2
        n0 = qh * NQ
        out[b, n0 : n0 + NQ, :] = res.results[core]["out"].astype(np.float32)
    return out
